# revision 17
# baseline (speedup 1.0000x reference)
"""Trainium2 Bass kernel for nn_MultiHeadAttention (b=4, s=2048, dim=1024, 16 heads).

Sharding: 8 cores = 4 batches x 2 head-groups. Core c handles batch c//2,
heads [8*(c%2), 8*(c%2)+8). Each core computes its QKV projection slice,
causal+padding-masked attention for its 8 heads, and a partial output
projection (W_o input-dim slice); the host sums the two head-group partials
per batch.

Device kernel per core (single Bass program, SPMD over 8 cores):
  phase 1: qkT = W_qk @ x^T (transposed layout, d on partitions)
           v   = x @ W_v^T  (natural layout, with a fused ones column)
  phase 2: per head: S^T[j,i] = k^T.T @ q^T tiles; exp on ScalarE with the
           key-padding mask as a per-partition bias; causal mask via
           gpsimd.affine_select; PV matmul with ones column producing both
           O^T[d,i] and the softmax denominator l[i]; normalize via
           reciprocal_approx_fast + partition_broadcast + tensor_mul.
  phase 3: y_partial = O @ W_o_slice^T accumulated over heads in PSUM.
"""

import numpy as np

import concourse.bass as bass
import concourse.mybir as mybir
import concourse.tile as tile
from concourse import bacc, library_config
from concourse.bass_utils import run_bass_kernel_spmd

# Problem shapes (hardcoded per contract)
B = 4
S = 2048
DIM = 1024
NH = 16
D = 64
N_CORES = 8
GROUPS = 2              # head groups (tensor-parallel dimension)
HPC = NH // GROUPS      # 8 heads per core
SCALE = D ** -0.5
MASK_BIAS = -30000.0    # additive logit bias for padded keys (exp underflows to 0)

JT = S // 128           # 16 key tiles of 128
NB = S // 512           # 4 query blocks of 512

F32 = mybir.dt.float32
BF16 = mybir.dt.bfloat16
IN_DT = BF16  # matmul operand dtype


def _mm(ap):
    return ap


DEBUG_DUMP = False


def _build_body(tc, xT, w_qkT, w_vT, w_oT, mask_bias, y, dumps=None):
    nc = tc.nc
    from contextlib import ExitStack

    # gpsimd ucode library providing InstPartitionBroadcast
    nc.gpsimd.load_library(library_config.attn)

    # ---- persistent SBUF tensors ----
    with ExitStack() as outer:
        persist = outer.enter_context(tc.tile_pool(name="persist", bufs=1))
        qk_sb = persist.tile([128, 8, S], IN_DT)       # [p, dimtile, tok]; tiles 0-3 q, 4-7 k
        v_sb = persist.tile([128, JT, HPC * 65], IN_DT)  # per tok-tile: 4 groups of 130 cols
        mb_sb = persist.tile([128, JT], F32)
        nc.sync.dma_start(out=mb_sb, in_=mask_bias[:, :])

        # ones columns of v_ext: group layout [evdims 0:64][ev1 64][oddims 65:129][od1 129]
        v_g = v_sb.rearrange("p t (g c) -> p t g c", c=130)
        nc.gpsimd.memset(v_g[:, :, :, 64:65], 1.0)
        nc.gpsimd.memset(v_g[:, :, :, 129:130], 1.0)

        # ================= phase 1: QKV projection =================
        with ExitStack() as ph1:
            wpool = ph1.enter_context(tc.tile_pool(name="w1", bufs=1))
            xpool = ph1.enter_context(tc.tile_pool(name="xq", bufs=2))
            pspool = ph1.enter_context(tc.tile_pool(name="ps1", bufs=4, space="PSUM"))

            w_qk_sb = wpool.tile([128, 8, 2 * HPC * D], IN_DT)   # [p, kt, 1024]
            w_v_sb = wpool.tile([128, 8, HPC * D], IN_DT)        # [p, kt, 512]
            nc.sync.dma_start(
                out=w_qk_sb, in_=w_qkT.rearrange("(kt p) j -> p kt j", p=128)
            )
            nc.sync.dma_start(
                out=w_v_sb, in_=w_vT.rearrange("(kt p) j -> p kt j", p=128)
            )

            xTr = xT.rearrange("(kt p) t -> p kt t", p=128)
            for q in range(4):  # token quarters of 512
                x_sb = xpool.tile([128, 8, 512], IN_DT, tag="x_sb")
                nc.sync.dma_start(out=x_sb, in_=xTr[:, :, 512 * q : 512 * q + 512])

                # qk^T: [qk-dim, tok]
                for dt in range(8):
                    ps = pspool.tile([128, 512], F32, tag="ps1")
                    for kt in range(8):
                        nc.tensor.matmul(
                            ps,
                            lhsT=_mm(w_qk_sb[:, kt, 128 * dt : 128 * dt + 128]),
                            rhs=_mm(x_sb[:, kt, :]),
                            start=(kt == 0),
                            stop=(kt == 7),
                        )
                    nc.scalar.copy(qk_sb[:, dt, 512 * q : 512 * q + 512], ps)

                # v natural: [tok, dh] -> strided into v_sb groups
                for tl in range(4):
                    tt = 4 * q + tl
                    ps = pspool.tile([128, 512], F32, tag="ps1")
                    for kt in range(8):
                        nc.tensor.matmul(
                            ps,
                            lhsT=_mm(x_sb[:, kt, 128 * tl : 128 * tl + 128]),
                            rhs=_mm(w_v_sb[:, kt, :]),
                            start=(kt == 0),
                            stop=(kt == 7),
                        )
                    psr = ps.rearrange("p (g two d) -> p g two d", two=2, d=64)
                    vr = v_g[:, tt]
                    nc.vector.tensor_copy(vr[:, :, 0:64], psr[:, :, 0, :])
                    nc.vector.tensor_copy(vr[:, :, 65:129], psr[:, :, 1, :])

        if dumps is not None:
            nc.sync.dma_start(out=dumps["qk"], in_=qk_sb)
            nc.sync.dma_start(out=dumps["v"], in_=v_sb)

        # ================= phases 2+3: o_sb lives here =================
        opool = outer.enter_context(tc.tile_pool(name="opool", bufs=1))
        o_sb = opool.tile([64, HPC, S], IN_DT)  # O^T per head [d, tok]

        # ================= phase 2: attention =================
        with ExitStack() as ph2:
            scpool = ph2.enter_context(tc.tile_pool(name="sc", bufs=4, space="PSUM"))
            pvpool = ph2.enter_context(tc.tile_pool(name="pv", bufs=2, space="PSUM"))
            expool = ph2.enter_context(tc.tile_pool(name="ex", bufs=6))
            npool = ph2.enter_context(tc.tile_pool(name="nrm", bufs=4))

            def v_ext(h, jt):
                g, par = h // 2, h % 2
                return v_g[:, jt, g, 65 * par : 65 * par + 65]

            def qT(h, sl):
                base = 64 * (h % 2)
                return qk_sb[base : base + 64, h // 2, sl]

            def kT(h, jt):
                base = 64 * (h % 2)
                return qk_sb[base : base + 64, 4 + h // 2, 128 * jt : 128 * jt + 128]

            for pair in range(HPC // 2):
                heads = (2 * pair, 2 * pair + 1)
                for ib in range(NB):  # query block of 512
                    pv = {h: pvpool.tile([65, 512], F32, tag=f"pv{hi}", name=f"pv{hi}")
                          for hi, h in enumerate(heads)}
                    last_jt = 4 * ib + 3
                    for jt in range(last_jt + 1):
                        diag = jt >= 4 * ib  # key tile intersects the diagonal
                        off = 128 * (jt - 4 * ib) if diag else 0
                        width = 512 - off
                        isl = slice(512 * ib + off, 512 * ib + 512)
                        for h in heads:
                            sc = scpool.tile([128, 512], F32, tag="sc")
                            nc.tensor.matmul(
                                sc[:, off:512],
                                lhsT=_mm(kT(h, jt)),
                                rhs=_mm(qT(h, isl)),
                                start=True,
                                stop=True,
                            )
                            ex = expool.tile([128, 512], IN_DT, tag="ex")
                            nc.scalar.activation(
                                ex[:, off:512],
                                sc[:, off:512],
                                mybir.ActivationFunctionType.Exp,
                                bias=mb_sb[:, jt : jt + 1],
                                scale=SCALE,
                            )
                            if diag:
                                # keep where (i - j) >= 0, else 0
                                nc.gpsimd.affine_select(
                                    out=ex[:, off:512],
                                    in_=ex[:, off:512],
                                    compare_op=mybir.AluOpType.is_ge,
                                    fill=0.0,
                                    base=0,
                                    pattern=[[1, width]],
                                    channel_multiplier=-1,
                                )
                            nc.tensor.matmul(
                                pv[h][:, off:512],
                                lhsT=_mm(v_ext(h, jt)),
                                rhs=_mm(ex[:, off:512]),
                                start=(jt == 0),
                                stop=(jt == last_jt),
                            )
                    # normalize: O = PV / l. l sits on psum partition 64.
                    # DVE lanes are partition-locked on HW: copy l to SBUF on
                    # lane 64, gpsimd-broadcast it down to partitions 0:64,
                    # then reciprocal+multiply at base 0.
                    for h in heads:
                        lsb = npool.tile([65, 512], F32, tag="lsb")
                        nc.vector.tensor_copy(lsb[64:65, :], pv[h][64:65, :])
                        # partition_broadcast reads partition 0 of the input
                        # tile (offset ignored on HW): DMA l down to a
                        # partition-0 tile first.
                        l0 = npool.tile([1, 512], F32, tag="l0")
                        nc.sync.dma_start(out=l0, in_=lsb[64:65, :])
                        braw = npool.tile([64, 512], F32, tag="braw")
                        nc.gpsimd.partition_broadcast(braw, l0)
                        bc = npool.tile([64, 512], F32, tag="bc")
                        nc.vector.reciprocal_approx_fast(bc, braw)
                        if dumps is not None and ib == 0 and h == heads[0] == 0:
                            nc.sync.dma_start(out=dumps["lsb"], in_=lsb)
                            nc.sync.dma_start(out=dumps["braw"], in_=braw)
                            nc.sync.dma_start(out=dumps["bc"], in_=bc)
                        nc.vector.tensor_mul(
                            o_sb[:, h, 512 * ib : 512 * ib + 512],
                            pv[h][0:64, :],
                            bc,
                        )

        if dumps is not None:
            nc.sync.dma_start(out=dumps["o"], in_=o_sb)

        # ================= phase 3: output projection =================
        with ExitStack() as ph3:
            wopool = ph3.enter_context(tc.tile_pool(name="wo", bufs=1))
            ypool = ph3.enter_context(tc.tile_pool(name="ysb", bufs=2))
            pspool3 = ph3.enter_context(tc.tile_pool(name="ps3", bufs=4, space="PSUM"))

            wo_sb = [
                wopool.tile([64, DIM], IN_DT, tag=f"wo{h}", name=f"wo{h}")
                for h in range(HPC)
            ]
            for h in range(HPC):
                nc.sync.dma_start(out=wo_sb[h], in_=w_oT[64 * h : 64 * h + 64, :])

            for tt in range(JT):
                ys = ypool.tile([128, DIM], F32, tag="ys")
                for eb in range(2):
                    ps = pspool3.tile([128, 512], F32, tag="ps3")
                    for h in range(HPC):
                        nc.tensor.matmul(
                            ps,
                            lhsT=_mm(o_sb[:, h, 128 * tt : 128 * tt + 128]),
                            rhs=_mm(wo_sb[h][:, 512 * eb : 512 * eb + 512]),
                            start=(h == 0),
                            stop=(h == HPC - 1),
                        )
                    nc.scalar.copy(ys[:, 512 * eb : 512 * eb + 512], ps)
                nc.sync.dma_start(out=y[128 * tt : 128 * tt + 128, :], in_=ys)


_PROGRAM_CACHE = {}


def build_program(debug_dump=False):
    key = ("nc", debug_dump)
    if key in _PROGRAM_CACHE:
        return _PROGRAM_CACHE[key]
    nc = bacc.Bacc(None, target_bir_lowering=False, debug=False)
    xT = nc.dram_tensor("xT", [DIM, S], IN_DT, kind="ExternalInput")
    w_qkT = nc.dram_tensor("w_qkT", [DIM, 2 * HPC * D], IN_DT, kind="ExternalInput")
    w_vT = nc.dram_tensor("w_vT", [DIM, HPC * D], IN_DT, kind="ExternalInput")
    w_oT = nc.dram_tensor("w_oT", [HPC * D, DIM], IN_DT, kind="ExternalInput")
    mask_bias = nc.dram_tensor("mask_bias", [128, JT], F32, kind="ExternalInput")
    y = nc.dram_tensor("y", [S, DIM], F32, kind="ExternalOutput")
    dumps = None
    if debug_dump:
        dumps = {
            "qk": nc.dram_tensor("qk_dump", [128, 8, S], IN_DT, kind="ExternalOutput")[:],
            "v": nc.dram_tensor("v_dump", [128, JT, HPC * 65], IN_DT, kind="ExternalOutput")[:],
            "o": nc.dram_tensor("o_dump", [64, HPC, S], IN_DT, kind="ExternalOutput")[:],
            "lsb": nc.dram_tensor("lsb_dump", [65, 512], F32, kind="ExternalOutput")[:],
            "braw": nc.dram_tensor("braw_dump", [64, 512], F32, kind="ExternalOutput")[:],
            "bc": nc.dram_tensor("bc_dump", [64, 512], F32, kind="ExternalOutput")[:],
        }
    with tile.TileContext(nc) as tc:
        _build_body(tc, xT[:], w_qkT[:], w_vT[:], w_oT[:], mask_bias[:], y[:], dumps)
    nc.compile()
    _PROGRAM_CACHE[key] = nc
    return nc


def make_in_maps(x, src_mask, W_qkv, W_o):
    import ml_dtypes

    np_in = ml_dtypes.bfloat16 if IN_DT == BF16 else np.float32
    x = np.asarray(x, dtype=np.float32)
    src_mask = np.asarray(src_mask)
    W_qkv = np.asarray(W_qkv, dtype=np.float32)
    W_o = np.asarray(W_o, dtype=np.float32)

    in_maps = []
    for c in range(N_CORES):
        b, g = c // GROUPS, c % GROUPS
        hw = HPC * D  # 512
        wq = W_qkv[g * hw : (g + 1) * hw]
        wk = W_qkv[DIM + g * hw : DIM + (g + 1) * hw]
        wv = W_qkv[2 * DIM + g * hw : 2 * DIM + (g + 1) * hw]
        mb = np.where(
            src_mask[b].reshape(JT, 128).T, np.float32(MASK_BIAS), np.float32(0.0)
        ).astype(np.float32)
        in_maps.append(
            {
                "xT": np.ascontiguousarray(x[b].T).astype(np_in),
                "w_qkT": np.ascontiguousarray(np.concatenate([wq, wk], 0).T).astype(
                    np_in
                ),
                "w_vT": np.ascontiguousarray(wv.T).astype(np_in),
                "w_oT": np.ascontiguousarray(
                    W_o[:, g * hw : (g + 1) * hw].T
                ).astype(np_in),
                "mask_bias": np.ascontiguousarray(mb),
            }
        )
    return in_maps


def run(x, src_mask, W_qkv, W_o, trace=False):
    nc = build_program()
    in_maps = make_in_maps(x, src_mask, W_qkv, W_o)
    res = run_bass_kernel_spmd(nc, in_maps, list(range(N_CORES)), trace=trace)
    parts = [res.results[c]["y"] for c in range(N_CORES)]
    out = np.empty((B, S, DIM), dtype=np.float32)
    for b in range(B):
        out[b] = parts[GROUPS * b] + parts[GROUPS * b + 1]
    return out, res


def kernel(x, src_mask, W_qkv, W_o):
    out, _ = run(x, src_mask, W_qkv, W_o, trace=False)
    return out


# revision 18
# speedup vs baseline: 1.4124x; 1.4124x over previous
"""Trainium2 Bass kernel for nn_MultiHeadAttention (b=4, s=2048, dim=1024, 16 heads).

Sharding: 8 cores = 4 batches x 2 head-groups. Core c handles batch c//2,
heads [8*(c%2), 8*(c%2)+8). Each core computes its QKV projection slice,
causal+padding-masked attention for its 8 heads, and a partial output
projection (W_o input-dim slice); the host sums the two head-group partials
per batch.

Device kernel per core (single Bass program, SPMD over 8 cores):
  phase 1: qkT = W_qk @ x^T (transposed layout, d on partitions)
           v   = x @ W_v^T  (natural layout, with a fused ones column)
  phase 2: per head: S^T[j,i] = k^T.T @ q^T tiles; exp on ScalarE with the
           key-padding mask as a per-partition bias; causal mask via
           gpsimd.affine_select; PV matmul with ones column producing both
           O^T[d,i] and the softmax denominator l[i]; normalize via
           reciprocal_approx_fast + partition_broadcast + tensor_mul.
  phase 3: y_partial = O @ W_o_slice^T accumulated over heads in PSUM.
"""

import numpy as np

import concourse.bass as bass
import concourse.mybir as mybir
import concourse.tile as tile
from concourse import bacc, library_config
from concourse.bass_utils import run_bass_kernel_spmd

# Problem shapes (hardcoded per contract)
B = 4
S = 2048
DIM = 1024
NH = 16
D = 64
N_CORES = 8
GROUPS = 2              # head groups (tensor-parallel dimension)
HPC = NH // GROUPS      # 8 heads per core
SCALE = D ** -0.5
MASK_BIAS = -30000.0    # additive logit bias for padded keys (exp underflows to 0)

JT = S // 128           # 16 key tiles of 128
NB = S // 512           # 4 query blocks of 512

F32 = mybir.dt.float32
BF16 = mybir.dt.bfloat16
IN_DT = BF16  # matmul operand dtype


def _mm(ap):
    return ap


DEBUG_DUMP = False


def _build_body(tc, xT, w_qkT, w_vT, w_oT, mask_bias, y, dumps=None):
    nc = tc.nc
    from contextlib import ExitStack

    # gpsimd ucode library providing InstPartitionBroadcast
    nc.gpsimd.load_library(library_config.attn)

    # ---- persistent SBUF tensors ----
    with ExitStack() as outer:
        persist = outer.enter_context(tc.tile_pool(name="persist", bufs=1))
        qk_sb = persist.tile([128, 8, S], IN_DT)       # [p, dimtile, tok]; tiles 0-3 q, 4-7 k
        # v_ext per head: [64 v-dims][ones][63 zeros] = 128 cols -> M=128 PV
        v_sb = persist.tile([128, JT, HPC * 128], IN_DT)
        mb_sb = persist.tile([128, JT], F32)
        nc.sync.dma_start(out=mb_sb, in_=mask_bias[:, :])

        v_g = v_sb.rearrange("p t (g c) -> p t g c", c=128)
        nc.gpsimd.memset(v_g[:, :, :, 64:65], 1.0)
        nc.gpsimd.memset(v_g[:, :, :, 65:128], 0.0)

        # ================= phase 1: QKV projection =================
        with ExitStack() as ph1:
            wpool = ph1.enter_context(tc.tile_pool(name="w1", bufs=1))
            xpool = ph1.enter_context(tc.tile_pool(name="xq", bufs=2))
            pspool = ph1.enter_context(tc.tile_pool(name="ps1", bufs=4, space="PSUM"))

            w_qk_sb = wpool.tile([128, 8, 2 * HPC * D], IN_DT)   # [p, kt, 1024]
            w_v_sb = wpool.tile([128, 8, HPC * D], IN_DT)        # [p, kt, 512]
            nc.sync.dma_start(
                out=w_qk_sb, in_=w_qkT.rearrange("(kt p) j -> p kt j", p=128)
            )
            nc.sync.dma_start(
                out=w_v_sb, in_=w_vT.rearrange("(kt p) j -> p kt j", p=128)
            )

            xTr = xT.rearrange("(kt p) t -> p kt t", p=128)
            for q in range(4):  # token quarters of 512
                x_sb = xpool.tile([128, 8, 512], IN_DT, tag="x_sb")
                nc.sync.dma_start(out=x_sb, in_=xTr[:, :, 512 * q : 512 * q + 512])

                # qk^T: [qk-dim, tok]
                for dt in range(8):
                    ps = pspool.tile([128, 512], F32, tag="ps1")
                    for kt in range(8):
                        nc.tensor.matmul(
                            ps,
                            lhsT=_mm(w_qk_sb[:, kt, 128 * dt : 128 * dt + 128]),
                            rhs=_mm(x_sb[:, kt, :]),
                            start=(kt == 0),
                            stop=(kt == 7),
                        )
                    nc.scalar.copy(qk_sb[:, dt, 512 * q : 512 * q + 512], ps)

                # v natural: [tok, dh] -> strided into v_sb groups
                for tl in range(4):
                    tt = 4 * q + tl
                    ps = pspool.tile([128, 512], F32, tag="ps1")
                    for kt in range(8):
                        nc.tensor.matmul(
                            ps,
                            lhsT=_mm(x_sb[:, kt, 128 * tl : 128 * tl + 128]),
                            rhs=_mm(w_v_sb[:, kt, :]),
                            start=(kt == 0),
                            stop=(kt == 7),
                        )
                    psr = ps.rearrange("p (g d) -> p g d", d=64)
                    nc.vector.tensor_copy(v_g[:, tt, :, 0:64], psr)

        if dumps is not None:
            nc.sync.dma_start(out=dumps["qk"], in_=qk_sb)
            nc.sync.dma_start(out=dumps["v"], in_=v_sb)

        # ===== phase 1.5: per-head zero-padded k tiles (K=128 scores) =====
        opool = outer.enter_context(tc.tile_pool(name="opool", bufs=1))
        # O^T stacked per head pair: rows 0:64 even head, 64:128 odd head
        o_pair = [
            opool.tile([128, S], IN_DT, tag=f"op{m}", name=f"op{m}")
            for m in range(HPC // 2)
        ]
        kp = [
            opool.tile([128, S], IN_DT, tag=f"kp{h}", name=f"kp{h}")
            for h in range(HPC)
        ]
        for h in range(HPC):
            base = 64 * (h % 2)
            nc.vector.tensor_copy(
                kp[h][base : base + 64, :], qk_sb[base : base + 64, 4 + h // 2, :]
            )
            zb = 64 - base  # the other half
            nc.vector.memset(kp[h][zb : zb + 64, :], 0.0)

        # ================= phase 2: attention =================
        with ExitStack() as ph2:
            scpool = ph2.enter_context(tc.tile_pool(name="sc", bufs=2, space="PSUM"))
            pvpool = ph2.enter_context(tc.tile_pool(name="pv", bufs=1, space="PSUM"))
            expool = ph2.enter_context(tc.tile_pool(name="ex", bufs=6))
            npool = ph2.enter_context(tc.tile_pool(name="nrm", bufs=3))

            def blocks(lo, hi):
                while lo < hi:
                    b = min((lo // 512 + 1) * 512, hi)
                    yield lo, b
                    lo = b

            for pair in range(HPC // 2):
                heads = (2 * pair, 2 * pair + 1)
                for ih in range(2):  # query halves of 1024
                    i_lo, i_hi = 1024 * ih, 1024 * (ih + 1)
                    pv = {
                        h: [
                            pvpool.tile(
                                [128, 512], F32, tag=f"pv{hi_}{b}", name=f"pv{hi_}{b}"
                            )
                            for b in range(2)
                        ]
                        for hi_, h in enumerate(heads)
                    }
                    for jt in range(8 * ih + 8):
                        j0 = 128 * jt
                        diag = j0 >= i_lo
                        c_lo = max(i_lo, j0)
                        c_off = c_lo - i_lo
                        for h in heads:
                            sc = scpool.tile([128, 1024], F32, tag="sc")
                            for lo, bhi in blocks(c_lo, i_hi):
                                nc.tensor.matmul(
                                    sc[:, lo - i_lo : bhi - i_lo],
                                    lhsT=kp[h][:, j0 : j0 + 128],
                                    rhs=qk_sb[:, pair, lo:bhi],
                                    start=True,
                                    stop=True,
                                )
                            ex = expool.tile([128, 1024], IN_DT, tag="ex")
                            nc.scalar.activation(
                                ex[:, c_off:1024],
                                sc[:, c_off:1024],
                                mybir.ActivationFunctionType.Exp,
                                bias=mb_sb[:, jt : jt + 1],
                                scale=SCALE,
                            )
                            if diag:
                                # causal boundary lives in the first 128 cols
                                nc.gpsimd.affine_select(
                                    out=ex[:, c_off : c_off + 128],
                                    in_=ex[:, c_off : c_off + 128],
                                    compare_op=mybir.AluOpType.is_ge,
                                    fill=0.0,
                                    base=0,
                                    pattern=[[1, 128]],
                                    channel_multiplier=-1,
                                )
                            for lo, bhi in blocks(c_lo, i_hi):
                                ib2 = (lo - i_lo) // 512
                                a0 = i_lo + 512 * ib2
                                nc.tensor.matmul(
                                    pv[h][ib2][:, lo - a0 : bhi - a0],
                                    lhsT=v_g[:, jt, h, :],
                                    rhs=ex[:, lo - i_lo : bhi - i_lo],
                                    start=(jt == 0),
                                    stop=(jt == 4 * (2 * ih + ib2) + 3),
                                )
                    # normalize: O = PV / l (l on psum partition 64; DVE lanes
                    # are partition-locked, gpsimd broadcast reads partition 0)
                    for hi_, h in enumerate(heads):
                        for ib2 in range(2):
                            acc = pv[h][ib2]
                            gl = i_lo + 512 * ib2
                            lsb = npool.tile([65, 512], F32, tag="lsb")
                            nc.vector.tensor_copy(lsb[64:65, :], acc[64:65, :])
                            l0 = npool.tile([1, 512], F32, tag="l0")
                            nc.sync.dma_start(out=l0, in_=lsb[64:65, :])
                            braw = npool.tile([64, 512], F32, tag="braw")
                            nc.gpsimd.partition_broadcast(braw, l0)
                            bc = npool.tile([64, 512], F32, tag="bc")
                            nc.vector.reciprocal_approx_fast(bc, braw)
                            if hi_ == 0:
                                nc.vector.tensor_mul(
                                    o_pair[pair][0:64, gl : gl + 512],
                                    acc[0:64, :],
                                    bc,
                                )
                            else:
                                ot = npool.tile([64, 512], IN_DT, tag="ot")
                                nc.vector.tensor_mul(ot, acc[0:64, :], bc)
                                nc.sync.dma_start(
                                    out=o_pair[pair][64:128, gl : gl + 512], in_=ot
                                )

        if dumps is not None:
            for m in range(HPC // 2):
                nc.sync.dma_start(out=dumps["o"][:, m, :], in_=o_pair[m])

        # ============ phase 3: output projection (K=128 stacked) ============
        with ExitStack() as ph3:
            wopool = ph3.enter_context(tc.tile_pool(name="wo", bufs=1))
            ypool = ph3.enter_context(tc.tile_pool(name="ysb", bufs=2))
            pspool3 = ph3.enter_context(tc.tile_pool(name="ps3", bufs=4, space="PSUM"))

            wo_sb = [
                wopool.tile([128, DIM], IN_DT, tag=f"wo{m}", name=f"wo{m}")
                for m in range(HPC // 2)
            ]
            for m in range(HPC // 2):
                nc.sync.dma_start(out=wo_sb[m], in_=w_oT[128 * m : 128 * m + 128, :])

            for tt in range(JT):
                ys = ypool.tile([128, DIM], F32, tag="ys")
                ps = [pspool3.tile([128, 512], F32, tag=f"ps3{e}", name=f"ps3{e}")
                      for e in range(2)]
                for m in range(HPC // 2):
                    for eb in range(2):
                        nc.tensor.matmul(
                            ps[eb],
                            lhsT=o_pair[m][:, 128 * tt : 128 * tt + 128],
                            rhs=wo_sb[m][:, 512 * eb : 512 * eb + 512],
                            start=(m == 0),
                            stop=(m == HPC // 2 - 1),
                        )
                for eb in range(2):
                    nc.scalar.copy(ys[:, 512 * eb : 512 * eb + 512], ps[eb])
                nc.sync.dma_start(out=y[128 * tt : 128 * tt + 128, :], in_=ys)


_PROGRAM_CACHE = {}


def build_program(debug_dump=False):
    key = ("nc", debug_dump)
    if key in _PROGRAM_CACHE:
        return _PROGRAM_CACHE[key]
    nc = bacc.Bacc(None, target_bir_lowering=False, debug=False)
    xT = nc.dram_tensor("xT", [DIM, S], IN_DT, kind="ExternalInput")
    w_qkT = nc.dram_tensor("w_qkT", [DIM, 2 * HPC * D], IN_DT, kind="ExternalInput")
    w_vT = nc.dram_tensor("w_vT", [DIM, HPC * D], IN_DT, kind="ExternalInput")
    w_oT = nc.dram_tensor("w_oT", [HPC * D, DIM], IN_DT, kind="ExternalInput")
    mask_bias = nc.dram_tensor("mask_bias", [128, JT], F32, kind="ExternalInput")
    y = nc.dram_tensor("y", [S, DIM], F32, kind="ExternalOutput")
    dumps = None
    if debug_dump:
        dumps = {
            "qk": nc.dram_tensor("qk_dump", [128, 8, S], IN_DT, kind="ExternalOutput")[:],
            "v": nc.dram_tensor("v_dump", [128, JT, HPC * 128], IN_DT, kind="ExternalOutput")[:],
            "o": nc.dram_tensor("o_dump", [128, HPC // 2, S], IN_DT, kind="ExternalOutput")[:],
        }
    with tile.TileContext(nc) as tc:
        _build_body(tc, xT[:], w_qkT[:], w_vT[:], w_oT[:], mask_bias[:], y[:], dumps)
    nc.compile()
    _PROGRAM_CACHE[key] = nc
    return nc


def make_in_maps(x, src_mask, W_qkv, W_o):
    import ml_dtypes

    np_in = ml_dtypes.bfloat16 if IN_DT == BF16 else np.float32
    x = np.asarray(x, dtype=np.float32)
    src_mask = np.asarray(src_mask)
    W_qkv = np.asarray(W_qkv, dtype=np.float32)
    W_o = np.asarray(W_o, dtype=np.float32)

    in_maps = []
    for c in range(N_CORES):
        b, g = c // GROUPS, c % GROUPS
        hw = HPC * D  # 512
        wq = W_qkv[g * hw : (g + 1) * hw]
        wk = W_qkv[DIM + g * hw : DIM + (g + 1) * hw]
        wv = W_qkv[2 * DIM + g * hw : 2 * DIM + (g + 1) * hw]
        mb = np.where(
            src_mask[b].reshape(JT, 128).T, np.float32(MASK_BIAS), np.float32(0.0)
        ).astype(np.float32)
        in_maps.append(
            {
                "xT": np.ascontiguousarray(x[b].T).astype(np_in),
                "w_qkT": np.ascontiguousarray(np.concatenate([wq, wk], 0).T).astype(
                    np_in
                ),
                "w_vT": np.ascontiguousarray(wv.T).astype(np_in),
                "w_oT": np.ascontiguousarray(
                    W_o[:, g * hw : (g + 1) * hw].T
                ).astype(np_in),
                "mask_bias": np.ascontiguousarray(mb),
            }
        )
    return in_maps


def run(x, src_mask, W_qkv, W_o, trace=False):
    nc = build_program()
    in_maps = make_in_maps(x, src_mask, W_qkv, W_o)
    res = run_bass_kernel_spmd(nc, in_maps, list(range(N_CORES)), trace=trace)
    parts = [res.results[c]["y"] for c in range(N_CORES)]
    out = np.empty((B, S, DIM), dtype=np.float32)
    for b in range(B):
        out[b] = parts[GROUPS * b] + parts[GROUPS * b + 1]
    return out, res


def kernel(x, src_mask, W_qkv, W_o):
    out, _ = run(x, src_mask, W_qkv, W_o, trace=False)
    return out


# revision 19
# speedup vs baseline: 1.4513x; 1.0275x over previous
"""Trainium2 Bass kernel for nn_MultiHeadAttention (b=4, s=2048, dim=1024, 16 heads).

Sharding: 8 cores = 4 batches x 2 head-groups. Core c handles batch c//2,
heads [8*(c%2), 8*(c%2)+8). Each core computes its QKV projection slice,
causal+padding-masked attention for its 8 heads, and a partial output
projection (W_o input-dim slice); the host sums the two head-group partials
per batch.

Device kernel per core (single Bass program, SPMD over 8 cores):
  phase 1: qkT = W_qk @ x^T (transposed layout, d on partitions)
           v   = x @ W_v^T  (natural layout, with a fused ones column)
  phase 2: per head: S^T[j,i] = k^T.T @ q^T tiles; exp on ScalarE with the
           key-padding mask as a per-partition bias; causal mask via
           gpsimd.affine_select; PV matmul with ones column producing both
           O^T[d,i] and the softmax denominator l[i]; normalize via
           reciprocal_approx_fast + partition_broadcast + tensor_mul.
  phase 3: y_partial = O @ W_o_slice^T accumulated over heads in PSUM.
"""

import numpy as np

import concourse.bass as bass
import concourse.mybir as mybir
import concourse.tile as tile
from concourse import bacc, library_config
from concourse.bass_utils import run_bass_kernel_spmd

# Problem shapes (hardcoded per contract)
B = 4
S = 2048
DIM = 1024
NH = 16
D = 64
N_CORES = 8
GROUPS = 2              # head groups (tensor-parallel dimension)
HPC = NH // GROUPS      # 8 heads per core
SCALE = D ** -0.5
MASK_BIAS = -30000.0    # additive logit bias for padded keys (exp underflows to 0)

JT = S // 128           # 16 key tiles of 128
NB = S // 512           # 4 query blocks of 512

F32 = mybir.dt.float32
BF16 = mybir.dt.bfloat16
IN_DT = BF16  # matmul operand dtype


def _mm(ap):
    return ap


DEBUG_DUMP = False


def _build_body(tc, xT, w_qkT, w_vT, w_oT, mask_bias, y, dumps=None):
    nc = tc.nc
    from contextlib import ExitStack

    # gpsimd ucode library providing InstPartitionBroadcast
    nc.gpsimd.load_library(library_config.attn)

    # ---- persistent SBUF tensors ----
    with ExitStack() as outer:
        persist = outer.enter_context(tc.tile_pool(name="persist", bufs=1))
        qk_sb = persist.tile([128, 8, S], IN_DT)       # [p, dimtile, tok]; tiles 0-3 q, 4-7 k
        # v_ext per head: [64 v-dims][ones][63 zeros] = 128 cols -> M=128 PV
        v_sb = persist.tile([128, JT, HPC * 128], IN_DT)
        mb_sb = persist.tile([128, JT], F32)
        nc.sync.dma_start(out=mb_sb, in_=mask_bias[:, :])

        v_g = v_sb.rearrange("p t (g c) -> p t g c", c=128)
        nc.gpsimd.memset(v_g[:, :, :, 64:65], 1.0)
        nc.gpsimd.memset(v_g[:, :, :, 65:128], 0.0)

        # causal mask tile: cmask[p, f] = 1 where f >= p else 0 (keep i-j >= 0)
        cmask = persist.tile([128, 128], IN_DT)
        nc.gpsimd.memset(cmask, 1.0)
        nc.gpsimd.affine_select(
            out=cmask,
            in_=cmask,
            compare_op=mybir.AluOpType.is_ge,
            fill=0.0,
            base=0,
            pattern=[[1, 128]],
            channel_multiplier=-1,
        )

        # ================= phase 1: QKV projection =================
        with ExitStack() as ph1:
            wpool = ph1.enter_context(tc.tile_pool(name="w1", bufs=1))
            xpool = ph1.enter_context(tc.tile_pool(name="xq", bufs=2))
            pspool = ph1.enter_context(tc.tile_pool(name="ps1", bufs=4, space="PSUM"))

            w_qk_sb = wpool.tile([128, 8, 2 * HPC * D], IN_DT)   # [p, kt, 1024]
            w_v_sb = wpool.tile([128, 8, HPC * D], IN_DT)        # [p, kt, 512]
            w_qkr = w_qkT.rearrange("(kt p) j -> p kt j", p=128)
            w_vr = w_vT.rearrange("(kt p) j -> p kt j", p=128)
            xTr = xT.rearrange("(kt p) t -> p kt t", p=128)

            x_first = xpool.tile([128, 8, 512], IN_DT, tag="x_sb")
            # interleave so the first matmul's operands land early
            nc.sync.dma_start(out=w_qk_sb[:, 0:2], in_=w_qkr[:, 0:2])
            nc.sync.dma_start(out=x_first[:, 0:4], in_=xTr[:, 0:4, 0:512])
            nc.sync.dma_start(out=w_qk_sb[:, 2:8], in_=w_qkr[:, 2:8])
            nc.sync.dma_start(out=x_first[:, 4:8], in_=xTr[:, 4:8, 0:512])
            nc.sync.dma_start(out=w_v_sb, in_=w_vr)

            for q in range(4):  # token quarters of 512
                if q == 0:
                    x_sb = x_first
                else:
                    x_sb = xpool.tile([128, 8, 512], IN_DT, tag="x_sb")
                    nc.sync.dma_start(
                        out=x_sb, in_=xTr[:, :, 512 * q : 512 * q + 512]
                    )

                # qk^T: [qk-dim, tok]
                for dt in range(8):
                    ps = pspool.tile([128, 512], F32, tag="ps1")
                    for kt in range(8):
                        nc.tensor.matmul(
                            ps,
                            lhsT=_mm(w_qk_sb[:, kt, 128 * dt : 128 * dt + 128]),
                            rhs=_mm(x_sb[:, kt, :]),
                            start=(kt == 0),
                            stop=(kt == 7),
                        )
                    nc.scalar.copy(qk_sb[:, dt, 512 * q : 512 * q + 512], ps)

                # v natural: [tok, dh] -> strided into v_sb groups
                for tl in range(4):
                    tt = 4 * q + tl
                    ps = pspool.tile([128, 512], F32, tag="ps1")
                    for kt in range(8):
                        nc.tensor.matmul(
                            ps,
                            lhsT=_mm(x_sb[:, kt, 128 * tl : 128 * tl + 128]),
                            rhs=_mm(w_v_sb[:, kt, :]),
                            start=(kt == 0),
                            stop=(kt == 7),
                        )
                    psr = ps.rearrange("p (g d) -> p g d", d=64)
                    nc.vector.tensor_copy(v_g[:, tt, :, 0:64], psr)

        if dumps is not None:
            nc.sync.dma_start(out=dumps["qk"], in_=qk_sb)
            nc.sync.dma_start(out=dumps["v"], in_=v_sb)

        # ===== phase 1.5: per-head zero-padded k tiles (K=128 scores) =====
        opool = outer.enter_context(tc.tile_pool(name="opool", bufs=1))
        # O^T stacked per head pair: rows 0:64 even head, 64:128 odd head
        o_pair = [
            opool.tile([128, S], IN_DT, tag=f"op{m}", name=f"op{m}")
            for m in range(HPC // 2)
        ]
        kp = [
            opool.tile([128, S], IN_DT, tag=f"kp{h}", name=f"kp{h}")
            for h in range(HPC)
        ]
        for h in range(HPC):
            base = 64 * (h % 2)
            nc.vector.tensor_copy(
                kp[h][base : base + 64, :], qk_sb[base : base + 64, 4 + h // 2, :]
            )
            zb = 64 - base  # the other half
            nc.vector.memset(kp[h][zb : zb + 64, :], 0.0)

        # ================= phase 2: attention =================
        with ExitStack() as ph2:
            scpool = ph2.enter_context(tc.tile_pool(name="sc", bufs=2, space="PSUM"))
            pvpool = ph2.enter_context(tc.tile_pool(name="pv", bufs=1, space="PSUM"))
            expool = ph2.enter_context(tc.tile_pool(name="ex", bufs=6))
            npool = ph2.enter_context(tc.tile_pool(name="nrm", bufs=3))

            def blocks(lo, hi):
                while lo < hi:
                    b = min((lo // 512 + 1) * 512, hi)
                    yield lo, b
                    lo = b

            for pair in range(HPC // 2):
                heads = (2 * pair, 2 * pair + 1)
                for ih in range(2):  # query halves of 1024
                    i_lo, i_hi = 1024 * ih, 1024 * (ih + 1)
                    pv = {
                        h: [
                            pvpool.tile(
                                [128, 512], F32, tag=f"pv{hi_}{b}", name=f"pv{hi_}{b}"
                            )
                            for b in range(2)
                        ]
                        for hi_, h in enumerate(heads)
                    }
                    for jt in range(8 * ih + 8):
                        j0 = 128 * jt
                        diag = j0 >= i_lo
                        c_lo = max(i_lo, j0)
                        c_off = c_lo - i_lo
                        for h in heads:
                            sc = scpool.tile([128, 1024], F32, tag="sc")
                            for lo, bhi in blocks(c_lo, i_hi):
                                nc.tensor.matmul(
                                    sc[:, lo - i_lo : bhi - i_lo],
                                    lhsT=kp[h][:, j0 : j0 + 128],
                                    rhs=qk_sb[:, pair, lo:bhi],
                                    start=True,
                                    stop=True,
                                )
                            ex = expool.tile([128, 1024], IN_DT, tag="ex")
                            nc.scalar.activation(
                                ex[:, c_off:1024],
                                sc[:, c_off:1024],
                                mybir.ActivationFunctionType.Exp,
                                bias=mb_sb[:, jt : jt + 1],
                                scale=SCALE,
                            )
                            if diag:
                                # causal boundary lives in the first 128 cols
                                nc.vector.tensor_mul(
                                    ex[:, c_off : c_off + 128],
                                    ex[:, c_off : c_off + 128],
                                    cmask,
                                )
                            for lo, bhi in blocks(c_lo, i_hi):
                                ib2 = (lo - i_lo) // 512
                                a0 = i_lo + 512 * ib2
                                nc.tensor.matmul(
                                    pv[h][ib2][:, lo - a0 : bhi - a0],
                                    lhsT=v_g[:, jt, h, :],
                                    rhs=ex[:, lo - i_lo : bhi - i_lo],
                                    start=(jt == 0),
                                    stop=(jt == 4 * (2 * ih + ib2) + 3),
                                )
                    # normalize: O = PV / l (l on psum partition 64; DVE lanes
                    # are partition-locked, gpsimd broadcast reads partition 0)
                    for hi_, h in enumerate(heads):
                        for ib2 in range(2):
                            acc = pv[h][ib2]
                            gl = i_lo + 512 * ib2
                            lsb = npool.tile([65, 512], F32, tag="lsb")
                            nc.vector.tensor_copy(lsb[64:65, :], acc[64:65, :])
                            l0 = npool.tile([1, 512], F32, tag="l0")
                            nc.sync.dma_start(out=l0, in_=lsb[64:65, :])
                            braw = npool.tile([64, 512], F32, tag="braw")
                            nc.gpsimd.partition_broadcast(braw, l0)
                            bc = npool.tile([64, 512], F32, tag="bc")
                            nc.vector.reciprocal_approx_fast(bc, braw)
                            if hi_ == 0:
                                nc.vector.tensor_mul(
                                    o_pair[pair][0:64, gl : gl + 512],
                                    acc[0:64, :],
                                    bc,
                                )
                            else:
                                ot = npool.tile([64, 512], IN_DT, tag="ot")
                                nc.vector.tensor_mul(ot, acc[0:64, :], bc)
                                nc.sync.dma_start(
                                    out=o_pair[pair][64:128, gl : gl + 512], in_=ot
                                )

        if dumps is not None:
            for m in range(HPC // 2):
                nc.sync.dma_start(out=dumps["o"][:, m, :], in_=o_pair[m])

        # ============ phase 3: output projection (K=128 stacked) ============
        with ExitStack() as ph3:
            wopool = ph3.enter_context(tc.tile_pool(name="wo", bufs=1))
            ypool = ph3.enter_context(tc.tile_pool(name="ysb", bufs=2))
            pspool3 = ph3.enter_context(tc.tile_pool(name="ps3", bufs=4, space="PSUM"))

            wo_sb = [
                wopool.tile([128, DIM], IN_DT, tag=f"wo{m}", name=f"wo{m}")
                for m in range(HPC // 2)
            ]
            for m in range(HPC // 2):
                nc.sync.dma_start(out=wo_sb[m], in_=w_oT[128 * m : 128 * m + 128, :])

            for tt in range(JT):
                ys = ypool.tile([128, DIM], F32, tag="ys")
                ps = [pspool3.tile([128, 512], F32, tag=f"ps3{e}", name=f"ps3{e}")
                      for e in range(2)]
                for m in range(HPC // 2):
                    for eb in range(2):
                        nc.tensor.matmul(
                            ps[eb],
                            lhsT=o_pair[m][:, 128 * tt : 128 * tt + 128],
                            rhs=wo_sb[m][:, 512 * eb : 512 * eb + 512],
                            start=(m == 0),
                            stop=(m == HPC // 2 - 1),
                        )
                for eb in range(2):
                    nc.scalar.copy(ys[:, 512 * eb : 512 * eb + 512], ps[eb])
                nc.sync.dma_start(out=y[128 * tt : 128 * tt + 128, :], in_=ys)


_PROGRAM_CACHE = {}


def build_program(debug_dump=False):
    key = ("nc", debug_dump)
    if key in _PROGRAM_CACHE:
        return _PROGRAM_CACHE[key]
    nc = bacc.Bacc(None, target_bir_lowering=False, debug=False)
    xT = nc.dram_tensor("xT", [DIM, S], IN_DT, kind="ExternalInput")
    w_qkT = nc.dram_tensor("w_qkT", [DIM, 2 * HPC * D], IN_DT, kind="ExternalInput")
    w_vT = nc.dram_tensor("w_vT", [DIM, HPC * D], IN_DT, kind="ExternalInput")
    w_oT = nc.dram_tensor("w_oT", [HPC * D, DIM], IN_DT, kind="ExternalInput")
    mask_bias = nc.dram_tensor("mask_bias", [128, JT], F32, kind="ExternalInput")
    y = nc.dram_tensor("y", [S, DIM], F32, kind="ExternalOutput")
    dumps = None
    if debug_dump:
        dumps = {
            "qk": nc.dram_tensor("qk_dump", [128, 8, S], IN_DT, kind="ExternalOutput")[:],
            "v": nc.dram_tensor("v_dump", [128, JT, HPC * 128], IN_DT, kind="ExternalOutput")[:],
            "o": nc.dram_tensor("o_dump", [128, HPC // 2, S], IN_DT, kind="ExternalOutput")[:],
        }
    with tile.TileContext(nc) as tc:
        _build_body(tc, xT[:], w_qkT[:], w_vT[:], w_oT[:], mask_bias[:], y[:], dumps)
    nc.compile()
    _PROGRAM_CACHE[key] = nc
    return nc


def make_in_maps(x, src_mask, W_qkv, W_o):
    import ml_dtypes

    np_in = ml_dtypes.bfloat16 if IN_DT == BF16 else np.float32
    x = np.asarray(x, dtype=np.float32)
    src_mask = np.asarray(src_mask)
    W_qkv = np.asarray(W_qkv, dtype=np.float32)
    W_o = np.asarray(W_o, dtype=np.float32)

    in_maps = []
    for c in range(N_CORES):
        b, g = c // GROUPS, c % GROUPS
        hw = HPC * D  # 512
        wq = W_qkv[g * hw : (g + 1) * hw]
        wk = W_qkv[DIM + g * hw : DIM + (g + 1) * hw]
        wv = W_qkv[2 * DIM + g * hw : 2 * DIM + (g + 1) * hw]
        mb = np.where(
            src_mask[b].reshape(JT, 128).T, np.float32(MASK_BIAS), np.float32(0.0)
        ).astype(np.float32)
        in_maps.append(
            {
                "xT": np.ascontiguousarray(x[b].T).astype(np_in),
                "w_qkT": np.ascontiguousarray(np.concatenate([wq, wk], 0).T).astype(
                    np_in
                ),
                "w_vT": np.ascontiguousarray(wv.T).astype(np_in),
                "w_oT": np.ascontiguousarray(
                    W_o[:, g * hw : (g + 1) * hw].T
                ).astype(np_in),
                "mask_bias": np.ascontiguousarray(mb),
            }
        )
    return in_maps


def run(x, src_mask, W_qkv, W_o, trace=False):
    nc = build_program()
    in_maps = make_in_maps(x, src_mask, W_qkv, W_o)
    res = run_bass_kernel_spmd(nc, in_maps, list(range(N_CORES)), trace=trace)
    parts = [res.results[c]["y"] for c in range(N_CORES)]
    out = np.empty((B, S, DIM), dtype=np.float32)
    for b in range(B):
        out[b] = parts[GROUPS * b] + parts[GROUPS * b + 1]
    return out, res


def kernel(x, src_mask, W_qkv, W_o):
    out, _ = run(x, src_mask, W_qkv, W_o, trace=False)
    return out


# revision 20
# speedup vs baseline: 1.5820x; 1.0901x over previous
"""Trainium2 Bass kernel for nn_MultiHeadAttention (b=4, s=2048, dim=1024, 16 heads).

Sharding: 8 cores = 4 batches x 2 head-groups. Core c handles batch c//2,
heads [8*(c%2), 8*(c%2)+8). Each core computes its QKV projection slice,
causal+padding-masked attention for its 8 heads, and a partial output
projection (W_o input-dim slice); the host sums the two head-group partials
per batch.

Device kernel per core (single Bass program, SPMD over 8 cores):
  phase 1: qkT = W_qk @ x^T (transposed layout, d on partitions)
           v   = x @ W_v^T  (natural layout, with a fused ones column)
  phase 2: per head: S^T[j,i] = k^T.T @ q^T tiles; exp on ScalarE with the
           key-padding mask as a per-partition bias; causal mask via
           gpsimd.affine_select; PV matmul with ones column producing both
           O^T[d,i] and the softmax denominator l[i]; normalize via
           reciprocal_approx_fast + partition_broadcast + tensor_mul.
  phase 3: y_partial = O @ W_o_slice^T accumulated over heads in PSUM.
"""

import numpy as np

import concourse.bass as bass
import concourse.mybir as mybir
import concourse.tile as tile
from concourse import bacc, library_config
from concourse.bass_utils import run_bass_kernel_spmd

# Problem shapes (hardcoded per contract)
B = 4
S = 2048
DIM = 1024
NH = 16
D = 64
N_CORES = 8
GROUPS = 2              # head groups (tensor-parallel dimension)
HPC = NH // GROUPS      # 8 heads per core
SCALE = D ** -0.5
MASK_BIAS = -30000.0    # additive logit bias for padded keys (exp underflows to 0)

JT = S // 128           # 16 key tiles of 128
NB = S // 512           # 4 query blocks of 512

F32 = mybir.dt.float32
BF16 = mybir.dt.bfloat16
IN_DT = BF16  # matmul operand dtype


def _mm(ap):
    return ap


DEBUG_DUMP = False


def _build_body(tc, xT, w_qkT, w_vT, w_oT, mask_bias, y, dumps=None):
    nc = tc.nc
    from contextlib import ExitStack

    # gpsimd ucode library providing InstPartitionBroadcast
    nc.gpsimd.load_library(library_config.attn)

    # ---- persistent SBUF tensors ----
    with ExitStack() as outer:
        persist = outer.enter_context(tc.tile_pool(name="persist", bufs=1))
        qk_sb = persist.tile([128, 8, S], IN_DT)       # [p, dimtile, tok]; tiles 0-3 q, 4-7 k
        # v_ext per head: [64 v-dims][ones][63 zeros] = 128 cols -> M=128 PV
        v_sb = persist.tile([128, JT, HPC * 128], IN_DT)
        mb_sb = persist.tile([128, JT], F32)
        nc.sync.dma_start(out=mb_sb, in_=mask_bias[:, :])

        v_g = v_sb.rearrange("p t (g c) -> p t g c", c=128)
        nc.gpsimd.memset(v_g[:, :, :, 64:65], 1.0)
        nc.gpsimd.memset(v_g[:, :, :, 65:128], 0.0)

        # causal mask tile: cmask[p, f] = 1 where f >= p else 0 (keep i-j >= 0)
        cmask = persist.tile([128, 128], IN_DT)
        nc.gpsimd.memset(cmask, 1.0)
        nc.gpsimd.affine_select(
            out=cmask,
            in_=cmask,
            compare_op=mybir.AluOpType.is_ge,
            fill=0.0,
            base=0,
            pattern=[[1, 128]],
            channel_multiplier=-1,
        )

        # pools living through phases 1.5-3
        opool = outer.enter_context(tc.tile_pool(name="opool", bufs=1))
        o_pair = [
            opool.tile([128, S], IN_DT, tag=f"op{m}", name=f"op{m}")
            for m in range(HPC // 2)
        ]
        kp = [
            opool.tile([128, S], IN_DT, tag=f"kp{h}", name=f"kp{h}")
            for h in range(HPC)
        ]
        for h in range(HPC):
            zb = 64 - 64 * (h % 2)
            nc.vector.memset(kp[h][zb : zb + 64, :], 0.0)
        wo_sb = [
            opool.tile([128, DIM], IN_DT, tag=f"wo{m}", name=f"wo{m}")
            for m in range(HPC // 2)
        ]

        # ================= phase 1: QKV projection =================
        with ExitStack() as ph1:
            wpool = ph1.enter_context(tc.tile_pool(name="w1", bufs=1))
            xpool = ph1.enter_context(tc.tile_pool(name="xq", bufs=2))
            pspool = ph1.enter_context(tc.tile_pool(name="ps1", bufs=4, space="PSUM"))

            w_qk_sb = wpool.tile([128, 8, 2 * HPC * D], IN_DT)   # [p, kt, 1024]
            w_v_sb = wpool.tile([128, 8, HPC * D], IN_DT)        # [p, kt, 512]
            w_qkr = w_qkT.rearrange("(kt p) j -> p kt j", p=128)
            w_vr = w_vT.rearrange("(kt p) j -> p kt j", p=128)
            xTr = xT.rearrange("(kt p) t -> p kt t", p=128)

            x_first = xpool.tile([128, 8, 512], IN_DT, tag="x_sb")
            # interleave so the first matmul's operands land early
            nc.sync.dma_start(out=w_qk_sb[:, 0:2], in_=w_qkr[:, 0:2])
            nc.sync.dma_start(out=x_first[:, 0:4], in_=xTr[:, 0:4, 0:512])
            nc.sync.dma_start(out=w_qk_sb[:, 2:8], in_=w_qkr[:, 2:8])
            nc.sync.dma_start(out=x_first[:, 4:8], in_=xTr[:, 4:8, 0:512])
            nc.sync.dma_start(out=w_v_sb, in_=w_vr)

            for q in range(4):  # token quarters of 512
                if q == 0:
                    x_sb = x_first
                else:
                    x_sb = xpool.tile([128, 8, 512], IN_DT, tag="x_sb")
                    nc.sync.dma_start(
                        out=x_sb, in_=xTr[:, :, 512 * q : 512 * q + 512]
                    )

                # qk^T: [qk-dim, tok]
                for dt in range(8):
                    ps = pspool.tile([128, 512], F32, tag="ps1")
                    for kt in range(8):
                        nc.tensor.matmul(
                            ps,
                            lhsT=_mm(w_qk_sb[:, kt, 128 * dt : 128 * dt + 128]),
                            rhs=_mm(x_sb[:, kt, :]),
                            start=(kt == 0),
                            stop=(kt == 7),
                        )
                    nc.scalar.copy(qk_sb[:, dt, 512 * q : 512 * q + 512], ps)

                # per-head zero-padded k slices for this quarter
                for h in range(HPC):
                    base = 64 * (h % 2)
                    nc.vector.tensor_copy(
                        kp[h][base : base + 64, 512 * q : 512 * q + 512],
                        qk_sb[base : base + 64, 4 + h // 2, 512 * q : 512 * q + 512],
                    )

                # v natural: [tok, dh] -> strided into v_sb groups
                for tl in range(4):
                    tt = 4 * q + tl
                    ps = pspool.tile([128, 512], F32, tag="ps1")
                    for kt in range(8):
                        nc.tensor.matmul(
                            ps,
                            lhsT=_mm(x_sb[:, kt, 128 * tl : 128 * tl + 128]),
                            rhs=_mm(w_v_sb[:, kt, :]),
                            start=(kt == 0),
                            stop=(kt == 7),
                        )
                    psr = ps.rearrange("p (g d) -> p g d", d=64)
                    nc.vector.tensor_copy(v_g[:, tt, :, 0:64], psr)

        if dumps is not None:
            nc.sync.dma_start(out=dumps["qk"], in_=qk_sb)
            nc.sync.dma_start(out=dumps["v"], in_=v_sb)

        # prefetch W_o into SBUF (sync queue, ahead of phase-2 traffic)
        for m in range(HPC // 2):
            nc.sync.dma_start(out=wo_sb[m], in_=w_oT[128 * m : 128 * m + 128, :])

        # ================= phase 2: attention =================
        with ExitStack() as ph2:
            scpool = ph2.enter_context(tc.tile_pool(name="sc", bufs=3, space="PSUM"))
            pvpool = ph2.enter_context(tc.tile_pool(name="pv", bufs=1, space="PSUM"))
            expool = ph2.enter_context(tc.tile_pool(name="ex", bufs=6))
            npool = ph2.enter_context(tc.tile_pool(name="nrm", bufs=3))

            def blocks(lo, hi):
                while lo < hi:
                    b = min((lo // 512 + 1) * 512, hi)
                    yield lo, b
                    lo = b

            for h in range(HPC):
                for ih in range(2):  # query halves of 1024
                    i_lo, i_hi = 1024 * ih, 1024 * (ih + 1)
                    pv = [
                        pvpool.tile([128, 512], F32, tag=f"pv{b}", name=f"pv{b}")
                        for b in range(2)
                    ]
                    for jt in range(8 * ih + 8):
                        j0 = 128 * jt
                        diag = j0 >= i_lo
                        c_lo = max(i_lo, j0)
                        c_off = c_lo - i_lo
                        sc = scpool.tile([128, 1024], F32, tag="sc")
                        for lo, bhi in blocks(c_lo, i_hi):
                            nc.tensor.matmul(
                                sc[:, lo - i_lo : bhi - i_lo],
                                lhsT=kp[h][:, j0 : j0 + 128],
                                rhs=qk_sb[:, h // 2, lo:bhi],
                                start=True,
                                stop=True,
                            )
                        ex = expool.tile([128, 1024], IN_DT, tag="ex")
                        nc.scalar.activation(
                            ex[:, c_off:1024],
                            sc[:, c_off:1024],
                            mybir.ActivationFunctionType.Exp,
                            bias=mb_sb[:, jt : jt + 1],
                            scale=SCALE,
                        )
                        if diag:
                            # causal boundary lives in the first 128 cols
                            nc.vector.tensor_mul(
                                ex[:, c_off : c_off + 128],
                                ex[:, c_off : c_off + 128],
                                cmask,
                            )
                        for lo, bhi in blocks(c_lo, i_hi):
                            ib2 = (lo - i_lo) // 512
                            a0 = i_lo + 512 * ib2
                            nc.tensor.matmul(
                                pv[ib2][:, lo - a0 : bhi - a0],
                                lhsT=v_g[:, jt, h, :],
                                rhs=ex[:, lo - i_lo : bhi - i_lo],
                                start=(jt == 0),
                                stop=(jt == 4 * (2 * ih + ib2) + 3),
                            )
                    # normalize: O = PV / l (l on psum partition 64; DVE lanes
                    # are partition-locked, gpsimd broadcast reads partition 0)
                    for ib2 in range(2):
                        if True:
                            acc = pv[ib2]
                            gl = i_lo + 512 * ib2
                            lsb = npool.tile([65, 512], F32, tag="lsb")
                            nc.vector.tensor_copy(lsb[64:65, :], acc[64:65, :])
                            l0 = npool.tile([1, 512], F32, tag="l0")
                            nc.sync.dma_start(out=l0, in_=lsb[64:65, :])
                            braw = npool.tile([64, 512], F32, tag="braw")
                            nc.gpsimd.partition_broadcast(braw, l0)
                            bc = npool.tile([64, 512], F32, tag="bc")
                            nc.vector.reciprocal_approx_fast(bc, braw)
                            if h % 2 == 0:
                                nc.vector.tensor_mul(
                                    o_pair[h // 2][0:64, gl : gl + 512],
                                    acc[0:64, :],
                                    bc,
                                )
                            else:
                                ot = npool.tile([64, 512], IN_DT, tag="ot")
                                nc.vector.tensor_mul(ot, acc[0:64, :], bc)
                                nc.sync.dma_start(
                                    out=o_pair[h // 2][64:128, gl : gl + 512], in_=ot
                                )

        if dumps is not None:
            for m in range(HPC // 2):
                nc.sync.dma_start(out=dumps["o"][:, m, :], in_=o_pair[m])

        # ============ phase 3: output projection (K=128 stacked) ============
        with ExitStack() as ph3:
            ypool = ph3.enter_context(tc.tile_pool(name="ysb", bufs=2))
            pspool3 = ph3.enter_context(tc.tile_pool(name="ps3", bufs=4, space="PSUM"))

            for tt in range(JT):
                ys = ypool.tile([128, DIM], F32, tag="ys")
                ps = [pspool3.tile([128, 512], F32, tag=f"ps3{e}", name=f"ps3{e}")
                      for e in range(2)]
                for m in range(HPC // 2):
                    for eb in range(2):
                        nc.tensor.matmul(
                            ps[eb],
                            lhsT=o_pair[m][:, 128 * tt : 128 * tt + 128],
                            rhs=wo_sb[m][:, 512 * eb : 512 * eb + 512],
                            start=(m == 0),
                            stop=(m == HPC // 2 - 1),
                        )
                for eb in range(2):
                    nc.scalar.copy(ys[:, 512 * eb : 512 * eb + 512], ps[eb])
                nc.sync.dma_start(out=y[128 * tt : 128 * tt + 128, :], in_=ys)


_PROGRAM_CACHE = {}


def build_program(debug_dump=False):
    key = ("nc", debug_dump)
    if key in _PROGRAM_CACHE:
        return _PROGRAM_CACHE[key]
    nc = bacc.Bacc(None, target_bir_lowering=False, debug=False)
    xT = nc.dram_tensor("xT", [DIM, S], IN_DT, kind="ExternalInput")
    w_qkT = nc.dram_tensor("w_qkT", [DIM, 2 * HPC * D], IN_DT, kind="ExternalInput")
    w_vT = nc.dram_tensor("w_vT", [DIM, HPC * D], IN_DT, kind="ExternalInput")
    w_oT = nc.dram_tensor("w_oT", [HPC * D, DIM], IN_DT, kind="ExternalInput")
    mask_bias = nc.dram_tensor("mask_bias", [128, JT], F32, kind="ExternalInput")
    y = nc.dram_tensor("y", [S, DIM], F32, kind="ExternalOutput")
    dumps = None
    if debug_dump:
        dumps = {
            "qk": nc.dram_tensor("qk_dump", [128, 8, S], IN_DT, kind="ExternalOutput")[:],
            "v": nc.dram_tensor("v_dump", [128, JT, HPC * 128], IN_DT, kind="ExternalOutput")[:],
            "o": nc.dram_tensor("o_dump", [128, HPC // 2, S], IN_DT, kind="ExternalOutput")[:],
        }
    with tile.TileContext(nc) as tc:
        _build_body(tc, xT[:], w_qkT[:], w_vT[:], w_oT[:], mask_bias[:], y[:], dumps)
    nc.compile()
    _PROGRAM_CACHE[key] = nc
    return nc


def make_in_maps(x, src_mask, W_qkv, W_o):
    import ml_dtypes

    np_in = ml_dtypes.bfloat16 if IN_DT == BF16 else np.float32
    x = np.asarray(x, dtype=np.float32)
    src_mask = np.asarray(src_mask)
    W_qkv = np.asarray(W_qkv, dtype=np.float32)
    W_o = np.asarray(W_o, dtype=np.float32)

    in_maps = []
    for c in range(N_CORES):
        b, g = c // GROUPS, c % GROUPS
        hw = HPC * D  # 512
        wq = W_qkv[g * hw : (g + 1) * hw]
        wk = W_qkv[DIM + g * hw : DIM + (g + 1) * hw]
        wv = W_qkv[2 * DIM + g * hw : 2 * DIM + (g + 1) * hw]
        mb = np.where(
            src_mask[b].reshape(JT, 128).T, np.float32(MASK_BIAS), np.float32(0.0)
        ).astype(np.float32)
        in_maps.append(
            {
                "xT": np.ascontiguousarray(x[b].T).astype(np_in),
                "w_qkT": np.ascontiguousarray(np.concatenate([wq, wk], 0).T).astype(
                    np_in
                ),
                "w_vT": np.ascontiguousarray(wv.T).astype(np_in),
                "w_oT": np.ascontiguousarray(
                    W_o[:, g * hw : (g + 1) * hw].T
                ).astype(np_in),
                "mask_bias": np.ascontiguousarray(mb),
            }
        )
    return in_maps


def run(x, src_mask, W_qkv, W_o, trace=False):
    nc = build_program()
    in_maps = make_in_maps(x, src_mask, W_qkv, W_o)
    res = run_bass_kernel_spmd(nc, in_maps, list(range(N_CORES)), trace=trace)
    parts = [res.results[c]["y"] for c in range(N_CORES)]
    out = np.empty((B, S, DIM), dtype=np.float32)
    for b in range(B):
        out[b] = parts[GROUPS * b] + parts[GROUPS * b + 1]
    return out, res


def kernel(x, src_mask, W_qkv, W_o):
    out, _ = run(x, src_mask, W_qkv, W_o, trace=False)
    return out


# revision 22
# speedup vs baseline: 1.5839x; 1.0012x over previous
"""Trainium2 Bass kernel for nn_MultiHeadAttention (b=4, s=2048, dim=1024, 16 heads).

Sharding: 8 cores = 4 batches x 2 head-groups. Core c handles batch c//2,
heads [8*(c%2), 8*(c%2)+8). Each core computes its QKV projection slice,
causal+padding-masked attention for its 8 heads, and a partial output
projection (W_o input-dim slice); the host sums the two head-group partials
per batch.

Device kernel per core (single Bass program, SPMD over 8 cores):
  phase 1: qkT = W_qk @ x^T (transposed layout, d on partitions)
           v   = x @ W_v^T  (natural layout, with a fused ones column)
  phase 2: per head: S^T[j,i] = k^T.T @ q^T tiles; exp on ScalarE with the
           key-padding mask as a per-partition bias; causal mask via
           gpsimd.affine_select; PV matmul with ones column producing both
           O^T[d,i] and the softmax denominator l[i]; normalize via
           reciprocal_approx_fast + partition_broadcast + tensor_mul.
  phase 3: y_partial = O @ W_o_slice^T accumulated over heads in PSUM.
"""

import numpy as np

import concourse.bass as bass
import concourse.mybir as mybir
import concourse.tile as tile
from concourse import bacc, library_config
from concourse.bass_utils import run_bass_kernel_spmd

# Problem shapes (hardcoded per contract)
B = 4
S = 2048
DIM = 1024
NH = 16
D = 64
N_CORES = 8
GROUPS = 2              # head groups (tensor-parallel dimension)
HPC = NH // GROUPS      # 8 heads per core
SCALE = D ** -0.5
MASK_BIAS = -30000.0    # additive logit bias for padded keys (exp underflows to 0)

JT = S // 128           # 16 key tiles of 128
NB = S // 512           # 4 query blocks of 512

F32 = mybir.dt.float32
BF16 = mybir.dt.bfloat16
IN_DT = BF16  # matmul operand dtype


def _mm(ap):
    return ap


DEBUG_DUMP = False


def _build_body(tc, xT, w_qkT, w_vT, w_oT, mask_bias, y, dumps=None):
    nc = tc.nc
    from contextlib import ExitStack

    # gpsimd ucode library providing InstPartitionBroadcast
    nc.gpsimd.load_library(library_config.attn)

    # ---- persistent SBUF tensors ----
    with ExitStack() as outer:
        persist = outer.enter_context(tc.tile_pool(name="persist", bufs=1))
        qk_sb = persist.tile([128, 8, S], IN_DT)       # [p, dimtile, tok]; tiles 0-3 q, 4-7 k
        # v_ext per head: [64 v-dims][ones][63 zeros] = 128 cols -> M=128 PV
        v_sb = persist.tile([128, JT, HPC * 128], IN_DT)
        mb_sb = persist.tile([128, JT], F32)
        nc.sync.dma_start(out=mb_sb, in_=mask_bias[:, :])

        v_g = v_sb.rearrange("p t (g c) -> p t g c", c=128)
        nc.gpsimd.memset(v_g[:, :, :, 64:65], 1.0)
        nc.gpsimd.memset(v_g[:, :, :, 65:128], 0.0)

        # causal mask tile: cmask[p, f] = 1 where f >= p else 0 (keep i-j >= 0)
        cmask = persist.tile([128, 128], IN_DT)
        nc.gpsimd.memset(cmask, 1.0)
        nc.gpsimd.affine_select(
            out=cmask,
            in_=cmask,
            compare_op=mybir.AluOpType.is_ge,
            fill=0.0,
            base=0,
            pattern=[[1, 128]],
            channel_multiplier=-1,
        )

        # pools living through phases 1.5-3
        opool = outer.enter_context(tc.tile_pool(name="opool", bufs=1))
        o_pair = [
            opool.tile([128, S], IN_DT, tag=f"op{m}", name=f"op{m}")
            for m in range(HPC // 2)
        ]
        kp = [
            opool.tile([128, S], IN_DT, tag=f"kp{h}", name=f"kp{h}")
            for h in range(HPC)
        ]
        for h in range(HPC):
            zb = 64 - 64 * (h % 2)
            nc.vector.memset(kp[h][zb : zb + 64, :], 0.0)
        wo_sb = [
            opool.tile([128, DIM], IN_DT, tag=f"wo{m}", name=f"wo{m}")
            for m in range(HPC // 2)
        ]

        # ================= phase 1: QKV projection =================
        with ExitStack() as ph1:
            wpool = ph1.enter_context(tc.tile_pool(name="w1", bufs=1))
            xpool = ph1.enter_context(tc.tile_pool(name="xq", bufs=2))
            pspool = ph1.enter_context(tc.tile_pool(name="ps1", bufs=4, space="PSUM"))

            w_qk_sb = wpool.tile([128, 8, 2 * HPC * D], IN_DT)   # [p, kt, 1024]
            w_v_sb = wpool.tile([128, 8, HPC * D], IN_DT)        # [p, kt, 512]
            w_qkr = w_qkT.rearrange("(kt p) j -> p kt j", p=128)
            w_vr = w_vT.rearrange("(kt p) j -> p kt j", p=128)
            xTr = xT.rearrange("(kt p) t -> p kt t", p=128)

            x_first = xpool.tile([128, 8, 512], IN_DT, tag="x_sb")
            # interleave so the first matmul's operands land early
            nc.sync.dma_start(out=w_qk_sb[:, 0:2], in_=w_qkr[:, 0:2])
            nc.sync.dma_start(out=x_first[:, 0:4], in_=xTr[:, 0:4, 0:512])
            nc.sync.dma_start(out=w_qk_sb[:, 2:8], in_=w_qkr[:, 2:8])
            nc.sync.dma_start(out=x_first[:, 4:8], in_=xTr[:, 4:8, 0:512])
            nc.sync.dma_start(out=w_v_sb, in_=w_vr)

            for q in range(4):  # token quarters of 512
                if q == 0:
                    x_sb = x_first
                else:
                    x_sb = xpool.tile([128, 8, 512], IN_DT, tag="x_sb")
                    nc.sync.dma_start(
                        out=x_sb, in_=xTr[:, :, 512 * q : 512 * q + 512]
                    )

                # qk^T: [qk-dim, tok]
                for dt in range(8):
                    ps = pspool.tile([128, 512], F32, tag="ps1")
                    for kt in range(8):
                        nc.tensor.matmul(
                            ps,
                            lhsT=_mm(w_qk_sb[:, kt, 128 * dt : 128 * dt + 128]),
                            rhs=_mm(x_sb[:, kt, :]),
                            start=(kt == 0),
                            stop=(kt == 7),
                        )
                    nc.scalar.copy(qk_sb[:, dt, 512 * q : 512 * q + 512], ps)

                # per-head zero-padded k slices for this quarter
                for h in range(HPC):
                    base = 64 * (h % 2)
                    nc.vector.tensor_copy(
                        kp[h][base : base + 64, 512 * q : 512 * q + 512],
                        qk_sb[base : base + 64, 4 + h // 2, 512 * q : 512 * q + 512],
                    )

                # v natural: [tok, dh] -> strided into v_sb groups
                for tl in range(4):
                    tt = 4 * q + tl
                    ps = pspool.tile([128, 512], F32, tag="ps1")
                    for kt in range(8):
                        nc.tensor.matmul(
                            ps,
                            lhsT=_mm(x_sb[:, kt, 128 * tl : 128 * tl + 128]),
                            rhs=_mm(w_v_sb[:, kt, :]),
                            start=(kt == 0),
                            stop=(kt == 7),
                        )
                    psr = ps.rearrange("p (g d) -> p g d", d=64)
                    nc.vector.tensor_copy(v_g[:, tt, :, 0:64], psr)

        if dumps is not None:
            nc.sync.dma_start(out=dumps["qk"], in_=qk_sb)
            nc.sync.dma_start(out=dumps["v"], in_=v_sb)

        # prefetch W_o into SBUF (sync queue, ahead of phase-2 traffic)
        for m in range(HPC // 2):
            nc.sync.dma_start(out=wo_sb[m], in_=w_oT[128 * m : 128 * m + 128, :])

        # ================= phase 2: attention =================
        with ExitStack() as ph2:
            scpool = ph2.enter_context(tc.tile_pool(name="sc", bufs=3, space="PSUM"))
            pvpool = ph2.enter_context(tc.tile_pool(name="pv", bufs=1, space="PSUM"))
            expool = ph2.enter_context(tc.tile_pool(name="ex", bufs=6))
            npool = ph2.enter_context(tc.tile_pool(name="nrm", bufs=3))

            def blocks(lo, hi):
                while lo < hi:
                    b = min((lo // 512 + 1) * 512, hi)
                    yield lo, b
                    lo = b

            # software pipeline: scores for unit u+1 are emitted before the
            # exp/PV of unit u, so the PE fills ACT's exp latency.
            units = [
                (h, ih, jt)
                for h in range(HPC)
                for ih in range(2)
                for jt in range(8 * ih + 8)
            ]
            pv_state = {}

            def emit_scores(u):
                h, ih, jt = u
                i_lo, i_hi = 1024 * ih, 1024 * (ih + 1)
                j0 = 128 * jt
                c_lo = max(i_lo, j0)
                sc = scpool.tile([128, 1024], F32, tag="sc", name="sc")
                for lo, bhi in blocks(c_lo, i_hi):
                    nc.tensor.matmul(
                        sc[:, lo - i_lo : bhi - i_lo],
                        lhsT=kp[h][:, j0 : j0 + 128],
                        rhs=qk_sb[:, h // 2, lo:bhi],
                        start=True,
                        stop=True,
                    )
                return sc

            def emit_consume(u, sc):
                h, ih, jt = u
                i_lo, i_hi = 1024 * ih, 1024 * (ih + 1)
                j0 = 128 * jt
                diag = j0 >= i_lo
                c_lo = max(i_lo, j0)
                c_off = c_lo - i_lo
                if jt == 0:
                    pv_state[(h, ih)] = [
                        pvpool.tile([128, 512], F32, tag=f"pv{b}", name=f"pv{b}")
                        for b in range(2)
                    ]
                pv = pv_state[(h, ih)]
                ex = expool.tile([128, 1024], IN_DT, tag="ex", name="ex")
                nc.scalar.activation(
                    ex[:, c_off:1024],
                    sc[:, c_off:1024],
                    mybir.ActivationFunctionType.Exp,
                    bias=mb_sb[:, jt : jt + 1],
                    scale=SCALE,
                )
                if diag:
                    # causal boundary lives in the first 128 cols
                    nc.vector.tensor_mul(
                        ex[:, c_off : c_off + 128],
                        ex[:, c_off : c_off + 128],
                        cmask,
                    )
                for lo, bhi in blocks(c_lo, i_hi):
                    ib2 = (lo - i_lo) // 512
                    a0 = i_lo + 512 * ib2
                    nc.tensor.matmul(
                        pv[ib2][:, lo - a0 : bhi - a0],
                        lhsT=v_g[:, jt, h, :],
                        rhs=ex[:, lo - i_lo : bhi - i_lo],
                        start=(jt == 0),
                        stop=(jt == 4 * (2 * ih + ib2) + 3),
                    )
                if jt == 8 * ih + 7:
                    emit_normalize(h, ih)

            def emit_normalize(h, ih):
                i_lo = 1024 * ih
                pv = pv_state.pop((h, ih))
                # normalize: O = PV / l (l on psum partition 64; DVE lanes are
                # partition-locked, gpsimd broadcast reads partition 0)
                for ib2 in range(2):
                    if True:
                        if True:
                            acc = pv[ib2]
                            gl = i_lo + 512 * ib2
                            lsb = npool.tile([65, 512], F32, tag="lsb")
                            nc.vector.tensor_copy(lsb[64:65, :], acc[64:65, :])
                            l0 = npool.tile([1, 512], F32, tag="l0")
                            nc.sync.dma_start(out=l0, in_=lsb[64:65, :])
                            braw = npool.tile([64, 512], F32, tag="braw")
                            nc.gpsimd.partition_broadcast(braw, l0)
                            bc = npool.tile([64, 512], F32, tag="bc")
                            nc.vector.reciprocal_approx_fast(bc, braw)
                            if h % 2 == 0:
                                nc.vector.tensor_mul(
                                    o_pair[h // 2][0:64, gl : gl + 512],
                                    acc[0:64, :],
                                    bc,
                                )
                            else:
                                ot = npool.tile([64, 512], IN_DT, tag="ot")
                                nc.vector.tensor_mul(ot, acc[0:64, :], bc)
                                nc.sync.dma_start(
                                    out=o_pair[h // 2][64:128, gl : gl + 512], in_=ot
                                )

            sc_next = emit_scores(units[0])
            for i in range(len(units)):
                sc_cur = sc_next
                if i + 1 < len(units):
                    sc_next = emit_scores(units[i + 1])
                emit_consume(units[i], sc_cur)

        if dumps is not None:
            for m in range(HPC // 2):
                nc.sync.dma_start(out=dumps["o"][:, m, :], in_=o_pair[m])

        # ============ phase 3: output projection (K=128 stacked) ============
        with ExitStack() as ph3:
            ypool = ph3.enter_context(tc.tile_pool(name="ysb", bufs=2))
            pspool3 = ph3.enter_context(tc.tile_pool(name="ps3", bufs=4, space="PSUM"))

            for tt in range(JT):
                ys = ypool.tile([128, DIM], F32, tag="ys")
                ps = [pspool3.tile([128, 512], F32, tag=f"ps3{e}", name=f"ps3{e}")
                      for e in range(2)]
                for m in range(HPC // 2):
                    for eb in range(2):
                        nc.tensor.matmul(
                            ps[eb],
                            lhsT=o_pair[m][:, 128 * tt : 128 * tt + 128],
                            rhs=wo_sb[m][:, 512 * eb : 512 * eb + 512],
                            start=(m == 0),
                            stop=(m == HPC // 2 - 1),
                        )
                for eb in range(2):
                    nc.scalar.copy(ys[:, 512 * eb : 512 * eb + 512], ps[eb])
                nc.sync.dma_start(out=y[128 * tt : 128 * tt + 128, :], in_=ys)


_PROGRAM_CACHE = {}


def build_program(debug_dump=False):
    key = ("nc", debug_dump)
    if key in _PROGRAM_CACHE:
        return _PROGRAM_CACHE[key]
    nc = bacc.Bacc(None, target_bir_lowering=False, debug=False)
    xT = nc.dram_tensor("xT", [DIM, S], IN_DT, kind="ExternalInput")
    w_qkT = nc.dram_tensor("w_qkT", [DIM, 2 * HPC * D], IN_DT, kind="ExternalInput")
    w_vT = nc.dram_tensor("w_vT", [DIM, HPC * D], IN_DT, kind="ExternalInput")
    w_oT = nc.dram_tensor("w_oT", [HPC * D, DIM], IN_DT, kind="ExternalInput")
    mask_bias = nc.dram_tensor("mask_bias", [128, JT], F32, kind="ExternalInput")
    y = nc.dram_tensor("y", [S, DIM], F32, kind="ExternalOutput")
    dumps = None
    if debug_dump:
        dumps = {
            "qk": nc.dram_tensor("qk_dump", [128, 8, S], IN_DT, kind="ExternalOutput")[:],
            "v": nc.dram_tensor("v_dump", [128, JT, HPC * 128], IN_DT, kind="ExternalOutput")[:],
            "o": nc.dram_tensor("o_dump", [128, HPC // 2, S], IN_DT, kind="ExternalOutput")[:],
        }
    with tile.TileContext(nc) as tc:
        _build_body(tc, xT[:], w_qkT[:], w_vT[:], w_oT[:], mask_bias[:], y[:], dumps)
    nc.compile()
    _PROGRAM_CACHE[key] = nc
    return nc


def make_in_maps(x, src_mask, W_qkv, W_o):
    import ml_dtypes

    np_in = ml_dtypes.bfloat16 if IN_DT == BF16 else np.float32
    x = np.asarray(x, dtype=np.float32)
    src_mask = np.asarray(src_mask)
    W_qkv = np.asarray(W_qkv, dtype=np.float32)
    W_o = np.asarray(W_o, dtype=np.float32)

    in_maps = []
    for c in range(N_CORES):
        b, g = c // GROUPS, c % GROUPS
        hw = HPC * D  # 512
        wq = W_qkv[g * hw : (g + 1) * hw]
        wk = W_qkv[DIM + g * hw : DIM + (g + 1) * hw]
        wv = W_qkv[2 * DIM + g * hw : 2 * DIM + (g + 1) * hw]
        mb = np.where(
            src_mask[b].reshape(JT, 128).T, np.float32(MASK_BIAS), np.float32(0.0)
        ).astype(np.float32)
        in_maps.append(
            {
                "xT": np.ascontiguousarray(x[b].T).astype(np_in),
                "w_qkT": np.ascontiguousarray(np.concatenate([wq, wk], 0).T).astype(
                    np_in
                ),
                "w_vT": np.ascontiguousarray(wv.T).astype(np_in),
                "w_oT": np.ascontiguousarray(
                    W_o[:, g * hw : (g + 1) * hw].T
                ).astype(np_in),
                "mask_bias": np.ascontiguousarray(mb),
            }
        )
    return in_maps


def run(x, src_mask, W_qkv, W_o, trace=False):
    nc = build_program()
    in_maps = make_in_maps(x, src_mask, W_qkv, W_o)
    res = run_bass_kernel_spmd(nc, in_maps, list(range(N_CORES)), trace=trace)
    parts = [res.results[c]["y"] for c in range(N_CORES)]
    out = np.empty((B, S, DIM), dtype=np.float32)
    for b in range(B):
        out[b] = parts[GROUPS * b] + parts[GROUPS * b + 1]
    return out, res


def kernel(x, src_mask, W_qkv, W_o):
    out, _ = run(x, src_mask, W_qkv, W_o, trace=False)
    return out


# revision 23
# speedup vs baseline: 1.5964x; 1.0079x over previous
"""Trainium2 Bass kernel for nn_MultiHeadAttention (b=4, s=2048, dim=1024, 16 heads).

Sharding: 8 cores = 4 batches x 2 head-groups. Core c handles batch c//2,
heads [8*(c%2), 8*(c%2)+8). Each core computes its QKV projection slice,
causal+padding-masked attention for its 8 heads, and a partial output
projection (W_o input-dim slice); the host sums the two head-group partials
per batch.

Device kernel per core (single Bass program, SPMD over 8 cores):
  phase 1: qkT = W_qk @ x^T (transposed layout, d on partitions)
           v   = x @ W_v^T  (natural layout, with a fused ones column)
  phase 2: per head: S^T[j,i] = k^T.T @ q^T tiles; exp on ScalarE with the
           key-padding mask as a per-partition bias; causal mask via
           gpsimd.affine_select; PV matmul with ones column producing both
           O^T[d,i] and the softmax denominator l[i]; normalize via
           reciprocal_approx_fast + partition_broadcast + tensor_mul.
  phase 3: y_partial = O @ W_o_slice^T accumulated over heads in PSUM.
"""

import numpy as np

import concourse.bass as bass
import concourse.mybir as mybir
import concourse.tile as tile
from concourse import bacc, library_config
from concourse.bass_utils import run_bass_kernel_spmd

# Problem shapes (hardcoded per contract)
B = 4
S = 2048
DIM = 1024
NH = 16
D = 64
N_CORES = 8
GROUPS = 2              # head groups (tensor-parallel dimension)
HPC = NH // GROUPS      # 8 heads per core
SCALE = D ** -0.5
MASK_BIAS = -30000.0    # additive logit bias for padded keys (exp underflows to 0)

JT = S // 128           # 16 key tiles of 128
NB = S // 512           # 4 query blocks of 512

F32 = mybir.dt.float32
BF16 = mybir.dt.bfloat16
IN_DT = BF16  # matmul operand dtype


def _mm(ap):
    return ap


DEBUG_DUMP = False


def _build_body(tc, xT, w_qkT, w_vT, w_oT, mask_bias, y, dumps=None):
    nc = tc.nc
    from contextlib import ExitStack

    # gpsimd ucode library providing InstPartitionBroadcast
    nc.gpsimd.load_library(library_config.attn)

    # ---- persistent SBUF tensors ----
    with ExitStack() as outer:
        persist = outer.enter_context(tc.tile_pool(name="persist", bufs=1))
        qk_sb = persist.tile([128, 8, S], IN_DT)       # [p, dimtile, tok]; tiles 0-3 q, 4-7 k
        # v_ext per head: [64 v-dims][ones][63 zeros] = 128 cols -> M=128 PV
        v_sb = persist.tile([128, JT, HPC * 128], IN_DT)
        mb_sb = persist.tile([128, JT], F32)
        nc.sync.dma_start(out=mb_sb, in_=mask_bias[:, :])

        v_g = v_sb.rearrange("p t (g c) -> p t g c", c=128)
        nc.gpsimd.memset(v_g[:, :, :, 64:65], 1.0)
        nc.gpsimd.memset(v_g[:, :, :, 65:128], 0.0)

        # causal mask tile: cmask[p, f] = 1 where f >= p else 0 (keep i-j >= 0)
        cmask = persist.tile([128, 128], IN_DT)
        nc.gpsimd.memset(cmask, 1.0)
        nc.gpsimd.affine_select(
            out=cmask,
            in_=cmask,
            compare_op=mybir.AluOpType.is_ge,
            fill=0.0,
            base=0,
            pattern=[[1, 128]],
            channel_multiplier=-1,
        )

        # pools living through phases 1.5-3
        opool = outer.enter_context(tc.tile_pool(name="opool", bufs=1))
        o_pair = [
            opool.tile([128, S], IN_DT, tag=f"op{m}", name=f"op{m}")
            for m in range(HPC // 2)
        ]
        kp = [
            opool.tile([128, S], IN_DT, tag=f"kp{h}", name=f"kp{h}")
            for h in range(HPC)
        ]
        for h in range(HPC):
            zb = 64 - 64 * (h % 2)
            nc.vector.memset(kp[h][zb : zb + 64, :], 0.0)
        wo_sb = [
            opool.tile([128, DIM], IN_DT, tag=f"wo{m}", name=f"wo{m}")
            for m in range(HPC // 2)
        ]

        # ================= phase 1: QKV projection =================
        with ExitStack() as ph1:
            wpool = ph1.enter_context(tc.tile_pool(name="w1", bufs=1))
            xpool = ph1.enter_context(tc.tile_pool(name="xq", bufs=2))
            pspool = ph1.enter_context(tc.tile_pool(name="ps1", bufs=4, space="PSUM"))

            w_qk_sb = wpool.tile([128, 8, 2 * HPC * D], IN_DT)   # [p, kt, 1024]
            w_v_sb = wpool.tile([128, 8, HPC * D], IN_DT)        # [p, kt, 512]
            w_qkr = w_qkT.rearrange("(kt p) j -> p kt j", p=128)
            w_vr = w_vT.rearrange("(kt p) j -> p kt j", p=128)
            xTr = xT.rearrange("(kt p) t -> p kt t", p=128)

            x_first = xpool.tile([128, 8, 512], IN_DT, tag="x_sb")
            # interleave so the first matmul's operands land early
            nc.sync.dma_start(out=w_qk_sb[:, 0:2], in_=w_qkr[:, 0:2])
            nc.sync.dma_start(out=x_first[:, 0:4], in_=xTr[:, 0:4, 0:512])
            nc.sync.dma_start(out=w_qk_sb[:, 2:8], in_=w_qkr[:, 2:8])
            nc.sync.dma_start(out=x_first[:, 4:8], in_=xTr[:, 4:8, 0:512])
            nc.sync.dma_start(out=w_v_sb, in_=w_vr)

            for q in range(4):  # token quarters of 512
                if q == 0:
                    x_sb = x_first
                else:
                    x_sb = xpool.tile([128, 8, 512], IN_DT, tag="x_sb")
                    nc.sync.dma_start(
                        out=x_sb, in_=xTr[:, :, 512 * q : 512 * q + 512]
                    )

                # qk^T: [qk-dim, tok]
                for dt in range(8):
                    ps = pspool.tile([128, 512], F32, tag="ps1")
                    for kt in range(8):
                        nc.tensor.matmul(
                            ps,
                            lhsT=_mm(w_qk_sb[:, kt, 128 * dt : 128 * dt + 128]),
                            rhs=_mm(x_sb[:, kt, :]),
                            start=(kt == 0),
                            stop=(kt == 7),
                        )
                    nc.scalar.copy(qk_sb[:, dt, 512 * q : 512 * q + 512], ps)

                # per-head zero-padded k slices for this quarter
                for h in range(HPC):
                    base = 64 * (h % 2)
                    nc.vector.tensor_copy(
                        kp[h][base : base + 64, 512 * q : 512 * q + 512],
                        qk_sb[base : base + 64, 4 + h // 2, 512 * q : 512 * q + 512],
                    )

                # v natural: [tok, dh] -> strided into v_sb groups
                for tl in range(4):
                    tt = 4 * q + tl
                    ps = pspool.tile([128, 512], F32, tag="ps1")
                    for kt in range(8):
                        nc.tensor.matmul(
                            ps,
                            lhsT=_mm(x_sb[:, kt, 128 * tl : 128 * tl + 128]),
                            rhs=_mm(w_v_sb[:, kt, :]),
                            start=(kt == 0),
                            stop=(kt == 7),
                        )
                    psr = ps.rearrange("p (g d) -> p g d", d=64)
                    nc.vector.tensor_copy(v_g[:, tt, :, 0:64], psr)

        if dumps is not None:
            nc.sync.dma_start(out=dumps["qk"], in_=qk_sb)
            nc.sync.dma_start(out=dumps["v"], in_=v_sb)

        # prefetch W_o into SBUF (sync queue, ahead of phase-2 traffic)
        for m in range(HPC // 2):
            nc.sync.dma_start(out=wo_sb[m], in_=w_oT[128 * m : 128 * m + 128, :])

        # ================= phase 2: attention =================
        with ExitStack() as ph2:
            scpool = ph2.enter_context(tc.tile_pool(name="sc", bufs=3, space="PSUM"))
            pvpool = ph2.enter_context(tc.tile_pool(name="pv", bufs=1, space="PSUM"))
            expool = ph2.enter_context(tc.tile_pool(name="ex", bufs=6))
            npool = ph2.enter_context(tc.tile_pool(name="nrm", bufs=3))

            def blocks(lo, hi):
                while lo < hi:
                    b = min((lo // 512 + 1) * 512, hi)
                    yield lo, b
                    lo = b

            # software pipeline: scores for unit u+1 are emitted before the
            # exp/PV of unit u, so the PE fills ACT's exp latency.
            units = [
                (h, ih, jt)
                for h in range(HPC)
                for ih in range(2)
                for jt in range(8 * ih + 8)
            ]
            pv_state = {}

            def emit_scores(u):
                h, ih, jt = u
                i_lo, i_hi = 1024 * ih, 1024 * (ih + 1)
                j0 = 128 * jt
                c_lo = max(i_lo, j0)
                sc = scpool.tile([128, 1024], F32, tag="sc", name="sc")
                for lo, bhi in blocks(c_lo, i_hi):
                    nc.tensor.matmul(
                        sc[:, lo - i_lo : bhi - i_lo],
                        lhsT=kp[h][:, j0 : j0 + 128],
                        rhs=qk_sb[:, h // 2, lo:bhi],
                        start=True,
                        stop=True,
                    )
                return sc

            def emit_consume(u, sc):
                h, ih, jt = u
                i_lo, i_hi = 1024 * ih, 1024 * (ih + 1)
                j0 = 128 * jt
                diag = j0 >= i_lo
                c_lo = max(i_lo, j0)
                c_off = c_lo - i_lo
                if jt == 0:
                    pv_state[(h, ih)] = [
                        pvpool.tile([128, 512], F32, tag=f"pv{b}", name=f"pv{b}")
                        for b in range(2)
                    ]
                pv = pv_state[(h, ih)]
                ex = expool.tile([128, 1024], IN_DT, tag="ex", name="ex")
                nc.scalar.activation(
                    ex[:, c_off:1024],
                    sc[:, c_off:1024],
                    mybir.ActivationFunctionType.Exp,
                    bias=mb_sb[:, jt : jt + 1],
                    scale=SCALE,
                )
                if diag:
                    # causal boundary lives in the first 128 cols
                    nc.vector.tensor_mul(
                        ex[:, c_off : c_off + 128],
                        ex[:, c_off : c_off + 128],
                        cmask,
                    )
                for lo, bhi in blocks(c_lo, i_hi):
                    ib2 = (lo - i_lo) // 512
                    a0 = i_lo + 512 * ib2
                    nc.tensor.matmul(
                        pv[ib2][:, lo - a0 : bhi - a0],
                        lhsT=v_g[:, jt, h, :],
                        rhs=ex[:, lo - i_lo : bhi - i_lo],
                        start=(jt == 0),
                        stop=(jt == 4 * (2 * ih + ib2) + 3),
                    )
                if jt == 8 * ih + 7:
                    emit_normalize(h, ih)

            def emit_normalize(h, ih):
                i_lo = 1024 * ih
                pv = pv_state.pop((h, ih))
                # normalize: O = PV / l (l on psum partition 64; DVE lanes are
                # partition-locked, gpsimd broadcast reads partition 0)
                for ib2 in range(2):
                    if True:
                        if True:
                            acc = pv[ib2]
                            gl = i_lo + 512 * ib2
                            lsb = npool.tile([65, 512], F32, tag="lsb")
                            nc.vector.tensor_copy(lsb[64:65, :], acc[64:65, :])
                            l0 = npool.tile([1, 512], F32, tag="l0")
                            nc.sync.dma_start(out=l0, in_=lsb[64:65, :])
                            braw = npool.tile([64, 512], F32, tag="braw")
                            nc.gpsimd.partition_broadcast(braw, l0)
                            bc = npool.tile([64, 512], F32, tag="bc")
                            nc.vector.reciprocal_approx_fast(bc, braw)
                            if h % 2 == 0:
                                nc.vector.tensor_mul(
                                    o_pair[h // 2][0:64, gl : gl + 512],
                                    acc[0:64, :],
                                    bc,
                                )
                            else:
                                ot = npool.tile([64, 512], IN_DT, tag="ot")
                                nc.vector.tensor_mul(ot, acc[0:64, :], bc)
                                nc.sync.dma_start(
                                    out=o_pair[h // 2][64:128, gl : gl + 512], in_=ot
                                )

            AHEAD = 2
            sc_q = [emit_scores(u) for u in units[:AHEAD]]
            for i in range(len(units)):
                if i + AHEAD < len(units):
                    sc_q.append(emit_scores(units[i + AHEAD]))
                emit_consume(units[i], sc_q.pop(0))

        if dumps is not None:
            for m in range(HPC // 2):
                nc.sync.dma_start(out=dumps["o"][:, m, :], in_=o_pair[m])

        # ============ phase 3: output projection (K=128 stacked) ============
        with ExitStack() as ph3:
            ypool = ph3.enter_context(tc.tile_pool(name="ysb", bufs=2))
            pspool3 = ph3.enter_context(tc.tile_pool(name="ps3", bufs=4, space="PSUM"))

            for tt in range(JT):
                ys = ypool.tile([128, DIM], F32, tag="ys")
                ps = [pspool3.tile([128, 512], F32, tag=f"ps3{e}", name=f"ps3{e}")
                      for e in range(2)]
                for m in range(HPC // 2):
                    for eb in range(2):
                        nc.tensor.matmul(
                            ps[eb],
                            lhsT=o_pair[m][:, 128 * tt : 128 * tt + 128],
                            rhs=wo_sb[m][:, 512 * eb : 512 * eb + 512],
                            start=(m == 0),
                            stop=(m == HPC // 2 - 1),
                        )
                for eb in range(2):
                    nc.scalar.copy(ys[:, 512 * eb : 512 * eb + 512], ps[eb])
                nc.sync.dma_start(out=y[128 * tt : 128 * tt + 128, :], in_=ys)


_PROGRAM_CACHE = {}


def build_program(debug_dump=False):
    key = ("nc", debug_dump)
    if key in _PROGRAM_CACHE:
        return _PROGRAM_CACHE[key]
    nc = bacc.Bacc(None, target_bir_lowering=False, debug=False)
    xT = nc.dram_tensor("xT", [DIM, S], IN_DT, kind="ExternalInput")
    w_qkT = nc.dram_tensor("w_qkT", [DIM, 2 * HPC * D], IN_DT, kind="ExternalInput")
    w_vT = nc.dram_tensor("w_vT", [DIM, HPC * D], IN_DT, kind="ExternalInput")
    w_oT = nc.dram_tensor("w_oT", [HPC * D, DIM], IN_DT, kind="ExternalInput")
    mask_bias = nc.dram_tensor("mask_bias", [128, JT], F32, kind="ExternalInput")
    y = nc.dram_tensor("y", [S, DIM], F32, kind="ExternalOutput")
    dumps = None
    if debug_dump:
        dumps = {
            "qk": nc.dram_tensor("qk_dump", [128, 8, S], IN_DT, kind="ExternalOutput")[:],
            "v": nc.dram_tensor("v_dump", [128, JT, HPC * 128], IN_DT, kind="ExternalOutput")[:],
            "o": nc.dram_tensor("o_dump", [128, HPC // 2, S], IN_DT, kind="ExternalOutput")[:],
        }
    with tile.TileContext(nc) as tc:
        _build_body(tc, xT[:], w_qkT[:], w_vT[:], w_oT[:], mask_bias[:], y[:], dumps)
    nc.compile()
    _PROGRAM_CACHE[key] = nc
    return nc


def make_in_maps(x, src_mask, W_qkv, W_o):
    import ml_dtypes

    np_in = ml_dtypes.bfloat16 if IN_DT == BF16 else np.float32
    x = np.asarray(x, dtype=np.float32)
    src_mask = np.asarray(src_mask)
    W_qkv = np.asarray(W_qkv, dtype=np.float32)
    W_o = np.asarray(W_o, dtype=np.float32)

    in_maps = []
    for c in range(N_CORES):
        b, g = c // GROUPS, c % GROUPS
        hw = HPC * D  # 512
        wq = W_qkv[g * hw : (g + 1) * hw]
        wk = W_qkv[DIM + g * hw : DIM + (g + 1) * hw]
        wv = W_qkv[2 * DIM + g * hw : 2 * DIM + (g + 1) * hw]
        mb = np.where(
            src_mask[b].reshape(JT, 128).T, np.float32(MASK_BIAS), np.float32(0.0)
        ).astype(np.float32)
        in_maps.append(
            {
                "xT": np.ascontiguousarray(x[b].T).astype(np_in),
                "w_qkT": np.ascontiguousarray(np.concatenate([wq, wk], 0).T).astype(
                    np_in
                ),
                "w_vT": np.ascontiguousarray(wv.T).astype(np_in),
                "w_oT": np.ascontiguousarray(
                    W_o[:, g * hw : (g + 1) * hw].T
                ).astype(np_in),
                "mask_bias": np.ascontiguousarray(mb),
            }
        )
    return in_maps


def run(x, src_mask, W_qkv, W_o, trace=False):
    nc = build_program()
    in_maps = make_in_maps(x, src_mask, W_qkv, W_o)
    res = run_bass_kernel_spmd(nc, in_maps, list(range(N_CORES)), trace=trace)
    parts = [res.results[c]["y"] for c in range(N_CORES)]
    out = np.empty((B, S, DIM), dtype=np.float32)
    for b in range(B):
        out[b] = parts[GROUPS * b] + parts[GROUPS * b + 1]
    return out, res


def kernel(x, src_mask, W_qkv, W_o):
    out, _ = run(x, src_mask, W_qkv, W_o, trace=False)
    return out


# revision 24
# speedup vs baseline: 1.6092x; 1.0080x over previous
"""Trainium2 Bass kernel for nn_MultiHeadAttention (b=4, s=2048, dim=1024, 16 heads).

Sharding: 8 cores = 4 batches x 2 head-groups. Core c handles batch c//2,
heads [8*(c%2), 8*(c%2)+8). Each core computes its QKV projection slice,
causal+padding-masked attention for its 8 heads, and a partial output
projection (W_o input-dim slice); the host sums the two head-group partials
per batch.

Device kernel per core (single Bass program, SPMD over 8 cores):
  phase 1: qkT = W_qk @ x^T (transposed layout, d on partitions)
           v   = x @ W_v^T  (natural layout, with a fused ones column)
  phase 2: per head: S^T[j,i] = k^T.T @ q^T tiles; exp on ScalarE with the
           key-padding mask as a per-partition bias; causal mask via
           gpsimd.affine_select; PV matmul with ones column producing both
           O^T[d,i] and the softmax denominator l[i]; normalize via
           reciprocal_approx_fast + partition_broadcast + tensor_mul.
  phase 3: y_partial = O @ W_o_slice^T accumulated over heads in PSUM.
"""

import numpy as np

import concourse.bass as bass
import concourse.mybir as mybir
import concourse.tile as tile
from concourse import bacc, library_config
from concourse.bass_utils import run_bass_kernel_spmd

# Problem shapes (hardcoded per contract)
B = 4
S = 2048
DIM = 1024
NH = 16
D = 64
N_CORES = 8
GROUPS = 2              # head groups (tensor-parallel dimension)
HPC = NH // GROUPS      # 8 heads per core
SCALE = D ** -0.5
MASK_BIAS = -30000.0    # additive logit bias for padded keys (exp underflows to 0)

JT = S // 128           # 16 key tiles of 128
NB = S // 512           # 4 query blocks of 512

F32 = mybir.dt.float32
BF16 = mybir.dt.bfloat16
IN_DT = BF16  # matmul operand dtype


def _mm(ap):
    return ap


DEBUG_DUMP = False


def _build_body(tc, xT, w_qkT, w_vT, w_oT, mask_bias, y, dumps=None):
    nc = tc.nc
    from contextlib import ExitStack

    # gpsimd ucode library providing InstPartitionBroadcast
    nc.gpsimd.load_library(library_config.attn)

    # ---- persistent SBUF tensors ----
    with ExitStack() as outer:
        persist = outer.enter_context(tc.tile_pool(name="persist", bufs=1))
        qk_sb = persist.tile([128, 8, S], IN_DT)       # [p, dimtile, tok]; tiles 0-3 q, 4-7 k
        # v_ext per head: [64 v-dims][ones][63 zeros] = 128 cols -> M=128 PV
        v_sb = persist.tile([128, JT, HPC * 128], IN_DT)
        mb_sb = persist.tile([128, JT], F32)
        nc.sync.dma_start(out=mb_sb, in_=mask_bias[:, :])

        v_g = v_sb.rearrange("p t (g c) -> p t g c", c=128)
        nc.gpsimd.memset(v_g[:, :, :, 64:65], 1.0)
        nc.gpsimd.memset(v_g[:, :, :, 65:128], 0.0)

        # causal mask tile: cmask[p, f] = 1 where f >= p else 0 (keep i-j >= 0)
        cmask = persist.tile([128, 128], IN_DT)
        nc.gpsimd.memset(cmask, 1.0)
        nc.gpsimd.affine_select(
            out=cmask,
            in_=cmask,
            compare_op=mybir.AluOpType.is_ge,
            fill=0.0,
            base=0,
            pattern=[[1, 128]],
            channel_multiplier=-1,
        )

        # pools living through phases 1.5-3
        opool = outer.enter_context(tc.tile_pool(name="opool", bufs=1))
        o_pair = [
            opool.tile([128, S], IN_DT, tag=f"op{m}", name=f"op{m}")
            for m in range(HPC // 2)
        ]
        kp = [
            opool.tile([128, S], IN_DT, tag=f"kp{h}", name=f"kp{h}")
            for h in range(HPC)
        ]
        for h in range(HPC):
            zb = 64 - 64 * (h % 2)
            nc.vector.memset(kp[h][zb : zb + 64, :], 0.0)
        wo_sb = [
            opool.tile([128, DIM], IN_DT, tag=f"wo{m}", name=f"wo{m}")
            for m in range(HPC // 2)
        ]

        # ================= phase 1: QKV projection =================
        with ExitStack() as ph1:
            wpool = ph1.enter_context(tc.tile_pool(name="w1", bufs=1))
            xpool = ph1.enter_context(tc.tile_pool(name="xq", bufs=2))
            pspool = ph1.enter_context(tc.tile_pool(name="ps1", bufs=4, space="PSUM"))

            w_qk_sb = wpool.tile([128, 8, 2 * HPC * D], IN_DT)   # [p, kt, 1024]
            w_v_sb = wpool.tile([128, 8, HPC * D], IN_DT)        # [p, kt, 512]
            w_qkr = w_qkT.rearrange("(kt p) j -> p kt j", p=128)
            w_vr = w_vT.rearrange("(kt p) j -> p kt j", p=128)
            xTr = xT.rearrange("(kt p) t -> p kt t", p=128)

            x_first = xpool.tile([128, 8, 512], IN_DT, tag="x_sb")
            # interleave so the first matmul's operands land early
            nc.sync.dma_start(out=w_qk_sb[:, 0:2], in_=w_qkr[:, 0:2])
            nc.sync.dma_start(out=x_first[:, 0:4], in_=xTr[:, 0:4, 0:512])
            nc.sync.dma_start(out=w_qk_sb[:, 2:8], in_=w_qkr[:, 2:8])
            nc.sync.dma_start(out=x_first[:, 4:8], in_=xTr[:, 4:8, 0:512])
            nc.sync.dma_start(out=w_v_sb, in_=w_vr)

            for q in range(4):  # token quarters of 512
                if q == 0:
                    x_sb = x_first
                else:
                    x_sb = xpool.tile([128, 8, 512], IN_DT, tag="x_sb")
                    nc.sync.dma_start(
                        out=x_sb, in_=xTr[:, :, 512 * q : 512 * q + 512]
                    )

                # qk^T: [qk-dim, tok]
                for dt in range(8):
                    ps = pspool.tile([128, 512], F32, tag="ps1")
                    for kt in range(8):
                        nc.tensor.matmul(
                            ps,
                            lhsT=_mm(w_qk_sb[:, kt, 128 * dt : 128 * dt + 128]),
                            rhs=_mm(x_sb[:, kt, :]),
                            start=(kt == 0),
                            stop=(kt == 7),
                        )
                    nc.scalar.copy(qk_sb[:, dt, 512 * q : 512 * q + 512], ps)

                # per-head zero-padded k slices for this quarter
                for h in range(HPC):
                    base = 64 * (h % 2)
                    nc.vector.tensor_copy(
                        kp[h][base : base + 64, 512 * q : 512 * q + 512],
                        qk_sb[base : base + 64, 4 + h // 2, 512 * q : 512 * q + 512],
                    )

                # v natural: [tok, dh] -> strided into v_sb groups
                for tl in range(4):
                    tt = 4 * q + tl
                    ps = pspool.tile([128, 512], F32, tag="ps1")
                    for kt in range(8):
                        nc.tensor.matmul(
                            ps,
                            lhsT=_mm(x_sb[:, kt, 128 * tl : 128 * tl + 128]),
                            rhs=_mm(w_v_sb[:, kt, :]),
                            start=(kt == 0),
                            stop=(kt == 7),
                        )
                    psr = ps.rearrange("p (g d) -> p g d", d=64)
                    nc.vector.tensor_copy(v_g[:, tt, :, 0:64], psr)

        if dumps is not None:
            nc.sync.dma_start(out=dumps["qk"], in_=qk_sb)
            nc.sync.dma_start(out=dumps["v"], in_=v_sb)

        # prefetch W_o into SBUF (sync queue, ahead of phase-2 traffic)
        for m in range(HPC // 2):
            nc.sync.dma_start(out=wo_sb[m], in_=w_oT[128 * m : 128 * m + 128, :])

        # ============ phase 2+3: attention + output projection ============
        with ExitStack() as ph2:
            scpool = ph2.enter_context(tc.tile_pool(name="sc", bufs=2, space="PSUM"))
            pvpool = ph2.enter_context(tc.tile_pool(name="pv", bufs=1, space="PSUM"))
            ps3pool = ph2.enter_context(tc.tile_pool(name="ps3", bufs=2, space="PSUM"))
            expool = ph2.enter_context(tc.tile_pool(name="ex", bufs=6))
            npool = ph2.enter_context(tc.tile_pool(name="nrm", bufs=3))
            ypool = ph2.enter_context(tc.tile_pool(name="ysb", bufs=2))

            def blocks(lo, hi):
                while lo < hi:
                    b = min((lo // 512 + 1) * 512, hi)
                    yield lo, b
                    lo = b

            # software pipeline: scores for unit u+1 are emitted before the
            # exp/PV of unit u, so the PE fills ACT's exp latency.
            units = [
                (h, ih, jt)
                for ih in range(2)
                for h in range(HPC)
                for jt in range(8 * ih + 8)
            ]
            pv_state = {}

            def emit_scores(u):
                h, ih, jt = u
                i_lo, i_hi = 1024 * ih, 1024 * (ih + 1)
                j0 = 128 * jt
                c_lo = max(i_lo, j0)
                sc = scpool.tile([128, 1024], F32, tag="sc", name="sc")
                for lo, bhi in blocks(c_lo, i_hi):
                    nc.tensor.matmul(
                        sc[:, lo - i_lo : bhi - i_lo],
                        lhsT=kp[h][:, j0 : j0 + 128],
                        rhs=qk_sb[:, h // 2, lo:bhi],
                        start=True,
                        stop=True,
                    )
                return sc

            def emit_consume(u, sc):
                h, ih, jt = u
                i_lo, i_hi = 1024 * ih, 1024 * (ih + 1)
                j0 = 128 * jt
                diag = j0 >= i_lo
                c_lo = max(i_lo, j0)
                c_off = c_lo - i_lo
                if jt == 0:
                    pv_state[(h, ih)] = [
                        pvpool.tile([128, 512], F32, tag=f"pv{b}", name=f"pv{b}")
                        for b in range(2)
                    ]
                pv = pv_state[(h, ih)]
                ex = expool.tile([128, 1024], IN_DT, tag="ex", name="ex")
                nc.scalar.activation(
                    ex[:, c_off:1024],
                    sc[:, c_off:1024],
                    mybir.ActivationFunctionType.Exp,
                    bias=mb_sb[:, jt : jt + 1],
                    scale=SCALE,
                )
                if diag:
                    # causal boundary lives in the first 128 cols
                    nc.vector.tensor_mul(
                        ex[:, c_off : c_off + 128],
                        ex[:, c_off : c_off + 128],
                        cmask,
                    )
                for lo, bhi in blocks(c_lo, i_hi):
                    ib2 = (lo - i_lo) // 512
                    a0 = i_lo + 512 * ib2
                    nc.tensor.matmul(
                        pv[ib2][:, lo - a0 : bhi - a0],
                        lhsT=v_g[:, jt, h, :],
                        rhs=ex[:, lo - i_lo : bhi - i_lo],
                        start=(jt == 0),
                        stop=(jt == 4 * (2 * ih + ib2) + 3),
                    )
                if jt == 8 * ih + 7:
                    emit_normalize(h, ih)

            def emit_normalize(h, ih):
                i_lo = 1024 * ih
                pv = pv_state.pop((h, ih))
                # normalize: O = PV / l (l on psum partition 64; DVE lanes are
                # partition-locked, gpsimd broadcast reads partition 0)
                for ib2 in range(2):
                    if True:
                        if True:
                            acc = pv[ib2]
                            gl = i_lo + 512 * ib2
                            lsb = npool.tile([65, 512], F32, tag="lsb")
                            nc.vector.tensor_copy(lsb[64:65, :], acc[64:65, :])
                            l0 = npool.tile([1, 512], F32, tag="l0")
                            nc.sync.dma_start(out=l0, in_=lsb[64:65, :])
                            braw = npool.tile([64, 512], F32, tag="braw")
                            nc.gpsimd.partition_broadcast(braw, l0)
                            bc = npool.tile([64, 512], F32, tag="bc")
                            nc.vector.reciprocal_approx_fast(bc, braw)
                            if h % 2 == 0:
                                nc.vector.tensor_mul(
                                    o_pair[h // 2][0:64, gl : gl + 512],
                                    acc[0:64, :],
                                    bc,
                                )
                            else:
                                ot = npool.tile([64, 512], IN_DT, tag="ot")
                                nc.vector.tensor_mul(ot, acc[0:64, :], bc)
                                nc.sync.dma_start(
                                    out=o_pair[h // 2][64:128, gl : gl + 512], in_=ot
                                )

            ys_state = {}

            def emit_wo(tt, eb):
                if eb == 0:
                    ys_state[tt] = ypool.tile([128, DIM], F32, tag="ys", name="ys")
                ys = ys_state[tt]
                ps = ps3pool.tile([128, 512], F32, tag="ps3", name="ps3")
                for m in range(HPC // 2):
                    nc.tensor.matmul(
                        ps,
                        lhsT=o_pair[m][:, 128 * tt : 128 * tt + 128],
                        rhs=wo_sb[m][:, 512 * eb : 512 * eb + 512],
                        start=(m == 0),
                        stop=(m == HPC // 2 - 1),
                    )
                nc.vector.tensor_copy(ys[:, 512 * eb : 512 * eb + 512], ps)
                if eb == 1:
                    nc.sync.dma_start(
                        out=y[128 * tt : 128 * tt + 128, :], in_=ys_state.pop(tt)
                    )

            # pipeline: scores one unit ahead; during the second query-half
            # (all heads' first-half O ready) interleave Wo chains for tt 0-7.
            n_ih0 = HPC * 8
            wo_sched = {}  # unit index -> list of (tt, eb)
            ih1_idxs = list(range(n_ih0, len(units)))
            first_half_chains = [(tt, eb) for tt in range(8) for eb in range(2)]
            step = max(1, len(ih1_idxs) // (len(first_half_chains) + 1))
            for ci, ch in enumerate(first_half_chains):
                idx = ih1_idxs[min((ci + 1) * step, len(ih1_idxs) - 1)]
                wo_sched.setdefault(idx, []).append(ch)

            sc_next = emit_scores(units[0])
            for i in range(len(units)):
                sc_cur = sc_next
                if i + 1 < len(units):
                    sc_next = emit_scores(units[i + 1])
                emit_consume(units[i], sc_cur)
                for tt, eb in wo_sched.get(i, ()):
                    emit_wo(tt, eb)

            for tt in range(8, JT):
                for eb in range(2):
                    emit_wo(tt, eb)

        if dumps is not None:
            for m in range(HPC // 2):
                nc.sync.dma_start(out=dumps["o"][:, m, :], in_=o_pair[m])


_PROGRAM_CACHE = {}


def build_program(debug_dump=False):
    key = ("nc", debug_dump)
    if key in _PROGRAM_CACHE:
        return _PROGRAM_CACHE[key]
    nc = bacc.Bacc(None, target_bir_lowering=False, debug=False)
    xT = nc.dram_tensor("xT", [DIM, S], IN_DT, kind="ExternalInput")
    w_qkT = nc.dram_tensor("w_qkT", [DIM, 2 * HPC * D], IN_DT, kind="ExternalInput")
    w_vT = nc.dram_tensor("w_vT", [DIM, HPC * D], IN_DT, kind="ExternalInput")
    w_oT = nc.dram_tensor("w_oT", [HPC * D, DIM], IN_DT, kind="ExternalInput")
    mask_bias = nc.dram_tensor("mask_bias", [128, JT], F32, kind="ExternalInput")
    y = nc.dram_tensor("y", [S, DIM], F32, kind="ExternalOutput")
    dumps = None
    if debug_dump:
        dumps = {
            "qk": nc.dram_tensor("qk_dump", [128, 8, S], IN_DT, kind="ExternalOutput")[:],
            "v": nc.dram_tensor("v_dump", [128, JT, HPC * 128], IN_DT, kind="ExternalOutput")[:],
            "o": nc.dram_tensor("o_dump", [128, HPC // 2, S], IN_DT, kind="ExternalOutput")[:],
        }
    with tile.TileContext(nc) as tc:
        _build_body(tc, xT[:], w_qkT[:], w_vT[:], w_oT[:], mask_bias[:], y[:], dumps)
    nc.compile()
    _PROGRAM_CACHE[key] = nc
    return nc


def make_in_maps(x, src_mask, W_qkv, W_o):
    import ml_dtypes

    np_in = ml_dtypes.bfloat16 if IN_DT == BF16 else np.float32
    x = np.asarray(x, dtype=np.float32)
    src_mask = np.asarray(src_mask)
    W_qkv = np.asarray(W_qkv, dtype=np.float32)
    W_o = np.asarray(W_o, dtype=np.float32)

    in_maps = []
    for c in range(N_CORES):
        b, g = c // GROUPS, c % GROUPS
        hw = HPC * D  # 512
        wq = W_qkv[g * hw : (g + 1) * hw]
        wk = W_qkv[DIM + g * hw : DIM + (g + 1) * hw]
        wv = W_qkv[2 * DIM + g * hw : 2 * DIM + (g + 1) * hw]
        mb = np.where(
            src_mask[b].reshape(JT, 128).T, np.float32(MASK_BIAS), np.float32(0.0)
        ).astype(np.float32)
        in_maps.append(
            {
                "xT": np.ascontiguousarray(x[b].T).astype(np_in),
                "w_qkT": np.ascontiguousarray(np.concatenate([wq, wk], 0).T).astype(
                    np_in
                ),
                "w_vT": np.ascontiguousarray(wv.T).astype(np_in),
                "w_oT": np.ascontiguousarray(
                    W_o[:, g * hw : (g + 1) * hw].T
                ).astype(np_in),
                "mask_bias": np.ascontiguousarray(mb),
            }
        )
    return in_maps


def run(x, src_mask, W_qkv, W_o, trace=False):
    nc = build_program()
    in_maps = make_in_maps(x, src_mask, W_qkv, W_o)
    res = run_bass_kernel_spmd(nc, in_maps, list(range(N_CORES)), trace=trace)
    parts = [res.results[c]["y"] for c in range(N_CORES)]
    out = np.empty((B, S, DIM), dtype=np.float32)
    for b in range(B):
        out[b] = parts[GROUPS * b] + parts[GROUPS * b + 1]
    return out, res


def kernel(x, src_mask, W_qkv, W_o):
    out, _ = run(x, src_mask, W_qkv, W_o, trace=False)
    return out


# revision 25
# speedup vs baseline: 1.6181x; 1.0055x over previous
"""Trainium2 Bass kernel for nn_MultiHeadAttention (b=4, s=2048, dim=1024, 16 heads).

Sharding: 8 cores = 4 batches x 2 head-groups. Core c handles batch c//2,
heads [8*(c%2), 8*(c%2)+8). Each core computes its QKV projection slice,
causal+padding-masked attention for its 8 heads, and a partial output
projection (W_o input-dim slice); the host sums the two head-group partials
per batch.

Device kernel per core (single Bass program, SPMD over 8 cores):
  phase 1: qkT = W_qk @ x^T (transposed layout, d on partitions)
           v   = x @ W_v^T  (natural layout, with a fused ones column)
  phase 2: per head: S^T[j,i] = k^T.T @ q^T tiles; exp on ScalarE with the
           key-padding mask as a per-partition bias; causal mask via
           gpsimd.affine_select; PV matmul with ones column producing both
           O^T[d,i] and the softmax denominator l[i]; normalize via
           reciprocal_approx_fast + partition_broadcast + tensor_mul.
  phase 3: y_partial = O @ W_o_slice^T accumulated over heads in PSUM.
"""

import numpy as np

import concourse.bass as bass
import concourse.mybir as mybir
import concourse.tile as tile
from concourse import bacc, library_config
from concourse.bass_utils import run_bass_kernel_spmd

# Problem shapes (hardcoded per contract)
B = 4
S = 2048
DIM = 1024
NH = 16
D = 64
N_CORES = 8
GROUPS = 2              # head groups (tensor-parallel dimension)
HPC = NH // GROUPS      # 8 heads per core
SCALE = D ** -0.5
MASK_BIAS = -30000.0    # additive logit bias for padded keys (exp underflows to 0)

JT = S // 128           # 16 key tiles of 128
NB = S // 512           # 4 query blocks of 512

F32 = mybir.dt.float32
BF16 = mybir.dt.bfloat16
IN_DT = BF16  # matmul operand dtype


def _mm(ap):
    return ap


DEBUG_DUMP = False


def _build_body(tc, xT, w_qkT, w_vT, w_oT, mask_bias, y, dumps=None):
    nc = tc.nc
    from contextlib import ExitStack

    # gpsimd ucode library providing InstPartitionBroadcast
    nc.gpsimd.load_library(library_config.attn)

    # ---- persistent SBUF tensors ----
    with ExitStack() as outer:
        persist = outer.enter_context(tc.tile_pool(name="persist", bufs=1))
        qk_sb = persist.tile([128, 8, S], IN_DT)       # [p, dimtile, tok]; tiles 0-3 q, 4-7 k
        # v_ext per head: [64 v-dims][ones][63 zeros] = 128 cols -> M=128 PV
        v_sb = persist.tile([128, JT, HPC * 128], IN_DT)
        mb_sb = persist.tile([128, JT], F32)
        nc.sync.dma_start(out=mb_sb, in_=mask_bias[:, :])

        v_g = v_sb.rearrange("p t (g c) -> p t g c", c=128)
        nc.gpsimd.memset(v_g[:, :, :, 64:65], 1.0)
        nc.gpsimd.memset(v_g[:, :, :, 65:128], 0.0)

        # causal mask tile: cmask[p, f] = 1 where f >= p else 0 (keep i-j >= 0)
        cmask = persist.tile([128, 128], IN_DT)
        nc.gpsimd.memset(cmask, 1.0)
        nc.gpsimd.affine_select(
            out=cmask,
            in_=cmask,
            compare_op=mybir.AluOpType.is_ge,
            fill=0.0,
            base=0,
            pattern=[[1, 128]],
            channel_multiplier=-1,
        )

        # pools living through phases 1.5-3
        opool = outer.enter_context(tc.tile_pool(name="opool", bufs=1))
        o_pair = [
            opool.tile([128, S], IN_DT, tag=f"op{m}", name=f"op{m}")
            for m in range(HPC // 2)
        ]
        kp = [
            opool.tile([128, S], IN_DT, tag=f"kp{h}", name=f"kp{h}")
            for h in range(HPC)
        ]
        for h in range(HPC):
            zb = 64 - 64 * (h % 2)
            nc.vector.memset(kp[h][zb : zb + 64, :], 0.0)
        wo_sb = [
            opool.tile([128, DIM], IN_DT, tag=f"wo{m}", name=f"wo{m}")
            for m in range(HPC // 2)
        ]

        # ================= phase 1: QKV projection =================
        with ExitStack() as ph1:
            wpool = ph1.enter_context(tc.tile_pool(name="w1", bufs=1))
            xpool = ph1.enter_context(tc.tile_pool(name="xq", bufs=2))
            pspool = ph1.enter_context(tc.tile_pool(name="ps1", bufs=4, space="PSUM"))

            w_qk_sb = wpool.tile([128, 8, 2 * HPC * D], IN_DT)   # [p, kt, 1024]
            w_v_sb = wpool.tile([128, 8, HPC * D], IN_DT)        # [p, kt, 512]
            w_qkr = w_qkT.rearrange("(kt p) j -> p kt j", p=128)
            w_vr = w_vT.rearrange("(kt p) j -> p kt j", p=128)
            xTr = xT.rearrange("(kt p) t -> p kt t", p=128)

            x_first = xpool.tile([128, 8, 512], IN_DT, tag="x_sb")
            # interleave so the first matmul's operands land early
            nc.sync.dma_start(out=w_qk_sb[:, 0:2], in_=w_qkr[:, 0:2])
            nc.sync.dma_start(out=x_first[:, 0:4], in_=xTr[:, 0:4, 0:512])
            nc.sync.dma_start(out=w_qk_sb[:, 2:8], in_=w_qkr[:, 2:8])
            nc.sync.dma_start(out=x_first[:, 4:8], in_=xTr[:, 4:8, 0:512])
            nc.sync.dma_start(out=w_v_sb, in_=w_vr)

            for q in range(4):  # token quarters of 512
                if q == 0:
                    x_sb = x_first
                else:
                    x_sb = xpool.tile([128, 8, 512], IN_DT, tag="x_sb")
                    nc.sync.dma_start(
                        out=x_sb, in_=xTr[:, :, 512 * q : 512 * q + 512]
                    )

                # qk^T: [qk-dim, tok]
                for dt in range(8):
                    ps = pspool.tile([128, 512], F32, tag="ps1")
                    for kt in range(8):
                        nc.tensor.matmul(
                            ps,
                            lhsT=_mm(w_qk_sb[:, kt, 128 * dt : 128 * dt + 128]),
                            rhs=_mm(x_sb[:, kt, :]),
                            start=(kt == 0),
                            stop=(kt == 7),
                        )
                    nc.scalar.copy(qk_sb[:, dt, 512 * q : 512 * q + 512], ps)

                # per-head zero-padded k slices for this quarter
                for h in range(HPC):
                    base = 64 * (h % 2)
                    nc.vector.tensor_copy(
                        kp[h][base : base + 64, 512 * q : 512 * q + 512],
                        qk_sb[base : base + 64, 4 + h // 2, 512 * q : 512 * q + 512],
                    )

                # v natural: [tok, dh] -> strided into v_sb groups
                for tl in range(4):
                    tt = 4 * q + tl
                    ps = pspool.tile([128, 512], F32, tag="ps1")
                    for kt in range(8):
                        nc.tensor.matmul(
                            ps,
                            lhsT=_mm(x_sb[:, kt, 128 * tl : 128 * tl + 128]),
                            rhs=_mm(w_v_sb[:, kt, :]),
                            start=(kt == 0),
                            stop=(kt == 7),
                        )
                    psr = ps.rearrange("p (g d) -> p g d", d=64)
                    nc.vector.tensor_copy(v_g[:, tt, :, 0:64], psr)

        if dumps is not None:
            nc.sync.dma_start(out=dumps["qk"], in_=qk_sb)
            nc.sync.dma_start(out=dumps["v"], in_=v_sb)

        # prefetch W_o into SBUF (sync queue, ahead of phase-2 traffic)
        for m in range(HPC // 2):
            nc.sync.dma_start(out=wo_sb[m], in_=w_oT[128 * m : 128 * m + 128, :])

        # ============ phase 2+3: attention + output projection ============
        with ExitStack() as ph2:
            scpool = ph2.enter_context(tc.tile_pool(name="sc", bufs=2, space="PSUM"))
            pvpool = ph2.enter_context(tc.tile_pool(name="pv", bufs=1, space="PSUM"))
            ps3pool = ph2.enter_context(tc.tile_pool(name="ps3", bufs=2, space="PSUM"))
            expool = ph2.enter_context(tc.tile_pool(name="ex", bufs=8))
            npool = ph2.enter_context(tc.tile_pool(name="nrm", bufs=4))
            ypool = ph2.enter_context(tc.tile_pool(name="ysb", bufs=2))

            def blocks(lo, hi):
                while lo < hi:
                    b = min((lo // 512 + 1) * 512, hi)
                    yield lo, b
                    lo = b

            # software pipeline: scores for unit u+1 are emitted before the
            # exp/PV of unit u, so the PE fills ACT's exp latency.
            units = [
                (h, ih, jt)
                for ih in range(2)
                for h in range(HPC)
                for jt in range(8 * ih + 8)
            ]
            pv_state = {}

            def emit_scores(u):
                h, ih, jt = u
                i_lo, i_hi = 1024 * ih, 1024 * (ih + 1)
                j0 = 128 * jt
                c_lo = max(i_lo, j0)
                sc = scpool.tile([128, 1024], F32, tag="sc", name="sc")
                for lo, bhi in blocks(c_lo, i_hi):
                    nc.tensor.matmul(
                        sc[:, lo - i_lo : bhi - i_lo],
                        lhsT=kp[h][:, j0 : j0 + 128],
                        rhs=qk_sb[:, h // 2, lo:bhi],
                        start=True,
                        stop=True,
                    )
                return sc

            def emit_consume(u, sc):
                h, ih, jt = u
                i_lo, i_hi = 1024 * ih, 1024 * (ih + 1)
                j0 = 128 * jt
                diag = j0 >= i_lo
                c_lo = max(i_lo, j0)
                c_off = c_lo - i_lo
                if jt == 0:
                    pv_state[(h, ih)] = [
                        pvpool.tile([128, 512], F32, tag=f"pv{b}", name=f"pv{b}")
                        for b in range(2)
                    ]
                pv = pv_state[(h, ih)]
                ex = expool.tile([128, 1024], IN_DT, tag="ex", name="ex")
                nc.scalar.activation(
                    ex[:, c_off:1024],
                    sc[:, c_off:1024],
                    mybir.ActivationFunctionType.Exp,
                    bias=mb_sb[:, jt : jt + 1],
                    scale=SCALE,
                )
                if diag:
                    # causal boundary lives in the first 128 cols
                    nc.vector.tensor_mul(
                        ex[:, c_off : c_off + 128],
                        ex[:, c_off : c_off + 128],
                        cmask,
                    )
                for lo, bhi in blocks(c_lo, i_hi):
                    ib2 = (lo - i_lo) // 512
                    a0 = i_lo + 512 * ib2
                    nc.tensor.matmul(
                        pv[ib2][:, lo - a0 : bhi - a0],
                        lhsT=v_g[:, jt, h, :],
                        rhs=ex[:, lo - i_lo : bhi - i_lo],
                        start=(jt == 0),
                        stop=(jt == 4 * (2 * ih + ib2) + 3),
                    )
                for ib2 in range(2):
                    if jt == 4 * (2 * ih + ib2) + 3:
                        emit_normalize(h, ih, ib2)
                if jt == 8 * ih + 7:
                    pv_state.pop((h, ih))

            def emit_normalize(h, ih, ib2):
                i_lo = 1024 * ih
                pv = pv_state[(h, ih)]
                # normalize: O = PV / l (l on psum partition 64; DVE lanes are
                # partition-locked, gpsimd broadcast reads partition 0)
                if True:
                    if True:
                        if True:
                            acc = pv[ib2]
                            gl = i_lo + 512 * ib2
                            lsb = npool.tile([65, 512], F32, tag="lsb")
                            nc.vector.tensor_copy(lsb[64:65, :], acc[64:65, :])
                            l0 = npool.tile([1, 512], F32, tag="l0")
                            nc.sync.dma_start(out=l0, in_=lsb[64:65, :])
                            braw = npool.tile([64, 512], F32, tag="braw")
                            nc.gpsimd.partition_broadcast(braw, l0)
                            bc = npool.tile([64, 512], F32, tag="bc")
                            nc.vector.reciprocal_approx_fast(bc, braw)
                            if h % 2 == 0:
                                nc.vector.tensor_mul(
                                    o_pair[h // 2][0:64, gl : gl + 512],
                                    acc[0:64, :],
                                    bc,
                                )
                            else:
                                ot = npool.tile([64, 512], IN_DT, tag="ot")
                                nc.vector.tensor_mul(ot, acc[0:64, :], bc)
                                nc.sync.dma_start(
                                    out=o_pair[h // 2][64:128, gl : gl + 512], in_=ot
                                )

            ys_state = {}

            def emit_wo(tt, eb):
                if eb == 0:
                    ys_state[tt] = ypool.tile([128, DIM], F32, tag="ys", name="ys")
                ys = ys_state[tt]
                ps = ps3pool.tile([128, 512], F32, tag="ps3", name="ps3")
                for m in range(HPC // 2):
                    nc.tensor.matmul(
                        ps,
                        lhsT=o_pair[m][:, 128 * tt : 128 * tt + 128],
                        rhs=wo_sb[m][:, 512 * eb : 512 * eb + 512],
                        start=(m == 0),
                        stop=(m == HPC // 2 - 1),
                    )
                nc.vector.tensor_copy(ys[:, 512 * eb : 512 * eb + 512], ps)
                if eb == 1:
                    nc.sync.dma_start(
                        out=y[128 * tt : 128 * tt + 128, :], in_=ys_state.pop(tt)
                    )

            # pipeline: scores one unit ahead. Wo chains interleave into the
            # second query-half: tt 0-7 once all heads' first-half O is ready,
            # tt 8-11 into the last head's final units (its ib2=0 block
            # normalizes at jt=11), tt 12-15 after the loop.
            n_ih0 = HPC * 8
            wo_sched = {}  # unit index -> list of (tt, eb)
            ih1_idxs = list(range(n_ih0, len(units)))
            first_half_chains = [(tt, eb) for tt in range(8) for eb in range(2)]
            step = max(1, (len(ih1_idxs) - 4) // (len(first_half_chains) + 1))
            for ci, ch in enumerate(first_half_chains):
                idx = ih1_idxs[min((ci + 1) * step, len(ih1_idxs) - 5)]
                wo_sched.setdefault(idx, []).append(ch)
            late = [(tt, eb) for tt in range(8, 12) for eb in range(2)]
            for ci, ch in enumerate(late):
                idx = ih1_idxs[-4 + min(ci // 2, 3)]
                wo_sched.setdefault(idx, []).append(ch)

            sc_next = emit_scores(units[0])
            for i in range(len(units)):
                sc_cur = sc_next
                if i + 1 < len(units):
                    sc_next = emit_scores(units[i + 1])
                emit_consume(units[i], sc_cur)
                for tt, eb in wo_sched.get(i, ()):
                    emit_wo(tt, eb)

            for tt in range(12, JT):
                for eb in range(2):
                    emit_wo(tt, eb)

        if dumps is not None:
            for m in range(HPC // 2):
                nc.sync.dma_start(out=dumps["o"][:, m, :], in_=o_pair[m])


_PROGRAM_CACHE = {}


def build_program(debug_dump=False):
    key = ("nc", debug_dump)
    if key in _PROGRAM_CACHE:
        return _PROGRAM_CACHE[key]
    nc = bacc.Bacc(None, target_bir_lowering=False, debug=False)
    xT = nc.dram_tensor("xT", [DIM, S], IN_DT, kind="ExternalInput")
    w_qkT = nc.dram_tensor("w_qkT", [DIM, 2 * HPC * D], IN_DT, kind="ExternalInput")
    w_vT = nc.dram_tensor("w_vT", [DIM, HPC * D], IN_DT, kind="ExternalInput")
    w_oT = nc.dram_tensor("w_oT", [HPC * D, DIM], IN_DT, kind="ExternalInput")
    mask_bias = nc.dram_tensor("mask_bias", [128, JT], F32, kind="ExternalInput")
    y = nc.dram_tensor("y", [S, DIM], F32, kind="ExternalOutput")
    dumps = None
    if debug_dump:
        dumps = {
            "qk": nc.dram_tensor("qk_dump", [128, 8, S], IN_DT, kind="ExternalOutput")[:],
            "v": nc.dram_tensor("v_dump", [128, JT, HPC * 128], IN_DT, kind="ExternalOutput")[:],
            "o": nc.dram_tensor("o_dump", [128, HPC // 2, S], IN_DT, kind="ExternalOutput")[:],
        }
    with tile.TileContext(nc) as tc:
        _build_body(tc, xT[:], w_qkT[:], w_vT[:], w_oT[:], mask_bias[:], y[:], dumps)
    nc.compile()
    _PROGRAM_CACHE[key] = nc
    return nc


def make_in_maps(x, src_mask, W_qkv, W_o):
    import ml_dtypes

    np_in = ml_dtypes.bfloat16 if IN_DT == BF16 else np.float32
    x = np.asarray(x, dtype=np.float32)
    src_mask = np.asarray(src_mask)
    W_qkv = np.asarray(W_qkv, dtype=np.float32)
    W_o = np.asarray(W_o, dtype=np.float32)

    in_maps = []
    for c in range(N_CORES):
        b, g = c // GROUPS, c % GROUPS
        hw = HPC * D  # 512
        wq = W_qkv[g * hw : (g + 1) * hw]
        wk = W_qkv[DIM + g * hw : DIM + (g + 1) * hw]
        wv = W_qkv[2 * DIM + g * hw : 2 * DIM + (g + 1) * hw]
        mb = np.where(
            src_mask[b].reshape(JT, 128).T, np.float32(MASK_BIAS), np.float32(0.0)
        ).astype(np.float32)
        in_maps.append(
            {
                "xT": np.ascontiguousarray(x[b].T).astype(np_in),
                "w_qkT": np.ascontiguousarray(np.concatenate([wq, wk], 0).T).astype(
                    np_in
                ),
                "w_vT": np.ascontiguousarray(wv.T).astype(np_in),
                "w_oT": np.ascontiguousarray(
                    W_o[:, g * hw : (g + 1) * hw].T
                ).astype(np_in),
                "mask_bias": np.ascontiguousarray(mb),
            }
        )
    return in_maps


def run(x, src_mask, W_qkv, W_o, trace=False):
    nc = build_program()
    in_maps = make_in_maps(x, src_mask, W_qkv, W_o)
    res = run_bass_kernel_spmd(nc, in_maps, list(range(N_CORES)), trace=trace)
    parts = [res.results[c]["y"] for c in range(N_CORES)]
    out = np.empty((B, S, DIM), dtype=np.float32)
    for b in range(B):
        out[b] = parts[GROUPS * b] + parts[GROUPS * b + 1]
    return out, res


def kernel(x, src_mask, W_qkv, W_o):
    out, _ = run(x, src_mask, W_qkv, W_o, trace=False)
    return out


# revision 26
# speedup vs baseline: 1.6185x; 1.0002x over previous
"""Trainium2 Bass kernel for nn_MultiHeadAttention (b=4, s=2048, dim=1024, 16 heads).

Sharding: 8 cores = 4 batches x 2 head-groups. Core c handles batch c//2,
heads [8*(c%2), 8*(c%2)+8). Each core computes its QKV projection slice,
causal+padding-masked attention for its 8 heads, and a partial output
projection (W_o input-dim slice); the host sums the two head-group partials
per batch.

Device kernel per core (single Bass program, SPMD over 8 cores):
  phase 1: qkT = W_qk @ x^T (transposed layout, d on partitions)
           v   = x @ W_v^T  (natural layout, with a fused ones column)
  phase 2: per head: S^T[j,i] = k^T.T @ q^T tiles; exp on ScalarE with the
           key-padding mask as a per-partition bias; causal mask via
           gpsimd.affine_select; PV matmul with ones column producing both
           O^T[d,i] and the softmax denominator l[i]; normalize via
           reciprocal_approx_fast + partition_broadcast + tensor_mul.
  phase 3: y_partial = O @ W_o_slice^T accumulated over heads in PSUM.
"""

import numpy as np

import concourse.bass as bass
import concourse.mybir as mybir
import concourse.tile as tile
from concourse import bacc, library_config
from concourse.bass_utils import run_bass_kernel_spmd

# Problem shapes (hardcoded per contract)
B = 4
S = 2048
DIM = 1024
NH = 16
D = 64
N_CORES = 8
GROUPS = 2              # head groups (tensor-parallel dimension)
HPC = NH // GROUPS      # 8 heads per core
SCALE = D ** -0.5
MASK_BIAS = -30000.0    # additive logit bias for padded keys (exp underflows to 0)

JT = S // 128           # 16 key tiles of 128
NB = S // 512           # 4 query blocks of 512

F32 = mybir.dt.float32
BF16 = mybir.dt.bfloat16
IN_DT = BF16  # matmul operand dtype


def _mm(ap):
    return ap


DEBUG_DUMP = False


def _build_body(tc, xT, w_qkT, w_vT, w_oT, mask_bias, y, dumps=None):
    nc = tc.nc
    from contextlib import ExitStack

    # gpsimd ucode library providing InstPartitionBroadcast
    nc.gpsimd.load_library(library_config.attn)

    # ---- persistent SBUF tensors ----
    with ExitStack() as outer:
        persist = outer.enter_context(tc.tile_pool(name="persist", bufs=1))
        qk_sb = persist.tile([128, 8, S], IN_DT)       # [p, dimtile, tok]; tiles 0-3 q, 4-7 k
        # v_ext per head: [64 v-dims][ones][63 zeros] = 128 cols -> M=128 PV
        v_sb = persist.tile([128, JT, HPC * 128], IN_DT)
        mb_sb = persist.tile([128, JT], F32)
        nc.sync.dma_start(out=mb_sb, in_=mask_bias[:, :])

        v_g = v_sb.rearrange("p t (g c) -> p t g c", c=128)
        nc.gpsimd.memset(v_g[:, :, :, 64:65], 1.0)
        nc.gpsimd.memset(v_g[:, :, :, 65:128], 0.0)

        # causal mask tile: cmask[p, f] = 1 where f >= p else 0 (keep i-j >= 0)
        cmask = persist.tile([128, 128], IN_DT)
        nc.gpsimd.memset(cmask, 1.0)
        nc.gpsimd.affine_select(
            out=cmask,
            in_=cmask,
            compare_op=mybir.AluOpType.is_ge,
            fill=0.0,
            base=0,
            pattern=[[1, 128]],
            channel_multiplier=-1,
        )

        # pools living through phases 1.5-3
        opool = outer.enter_context(tc.tile_pool(name="opool", bufs=1))
        o_pair = [
            opool.tile([128, S], IN_DT, tag=f"op{m}", name=f"op{m}")
            for m in range(HPC // 2)
        ]
        kp = [
            opool.tile([128, S], IN_DT, tag=f"kp{h}", name=f"kp{h}")
            for h in range(HPC)
        ]
        for h in range(HPC):
            zb = 64 - 64 * (h % 2)
            nc.vector.memset(kp[h][zb : zb + 64, :], 0.0)
        wo_sb = [
            opool.tile([128, DIM], IN_DT, tag=f"wo{m}", name=f"wo{m}")
            for m in range(HPC // 2)
        ]

        # ================= phase 1: QKV projection =================
        with ExitStack() as ph1:
            wpool = ph1.enter_context(tc.tile_pool(name="w1", bufs=1))
            xpool = ph1.enter_context(tc.tile_pool(name="xq", bufs=2))
            pspool = ph1.enter_context(tc.tile_pool(name="ps1", bufs=4, space="PSUM"))

            w_qk_sb = wpool.tile([128, 8, 2 * HPC * D], IN_DT)   # [p, kt, 1024]
            w_v_sb = wpool.tile([128, 8, HPC * D], IN_DT)        # [p, kt, 512]
            w_qkr = w_qkT.rearrange("(kt p) j -> p kt j", p=128)
            w_vr = w_vT.rearrange("(kt p) j -> p kt j", p=128)
            xTr = xT.rearrange("(kt p) t -> p kt t", p=128)

            x_first = xpool.tile([128, 8, 512], IN_DT, tag="x_sb")
            # interleave so the first matmul's operands land early
            nc.sync.dma_start(out=w_qk_sb[:, 0:2], in_=w_qkr[:, 0:2])
            nc.sync.dma_start(out=x_first[:, 0:4], in_=xTr[:, 0:4, 0:512])
            nc.sync.dma_start(out=w_qk_sb[:, 2:8], in_=w_qkr[:, 2:8])
            nc.sync.dma_start(out=x_first[:, 4:8], in_=xTr[:, 4:8, 0:512])
            nc.sync.dma_start(out=w_v_sb, in_=w_vr)

            for q in range(4):  # token quarters of 512
                if q == 0:
                    x_sb = x_first
                else:
                    x_sb = xpool.tile([128, 8, 512], IN_DT, tag="x_sb")
                    nc.sync.dma_start(
                        out=x_sb, in_=xTr[:, :, 512 * q : 512 * q + 512]
                    )

                # qk^T: [qk-dim, tok]
                for dt in range(8):
                    ps = pspool.tile([128, 512], F32, tag="ps1")
                    for kt in range(8):
                        nc.tensor.matmul(
                            ps,
                            lhsT=_mm(w_qk_sb[:, kt, 128 * dt : 128 * dt + 128]),
                            rhs=_mm(x_sb[:, kt, :]),
                            start=(kt == 0),
                            stop=(kt == 7),
                        )
                    nc.scalar.copy(qk_sb[:, dt, 512 * q : 512 * q + 512], ps)

                # per-head zero-padded k slices for this quarter
                for h in range(HPC):
                    base = 64 * (h % 2)
                    nc.vector.tensor_copy(
                        kp[h][base : base + 64, 512 * q : 512 * q + 512],
                        qk_sb[base : base + 64, 4 + h // 2, 512 * q : 512 * q + 512],
                    )

                # v natural: [tok, dh] -> strided into v_sb groups
                for tl in range(4):
                    tt = 4 * q + tl
                    ps = pspool.tile([128, 512], F32, tag="ps1")
                    for kt in range(8):
                        nc.tensor.matmul(
                            ps,
                            lhsT=_mm(x_sb[:, kt, 128 * tl : 128 * tl + 128]),
                            rhs=_mm(w_v_sb[:, kt, :]),
                            start=(kt == 0),
                            stop=(kt == 7),
                        )
                    psr = ps.rearrange("p (g d) -> p g d", d=64)
                    nc.vector.tensor_copy(v_g[:, tt, :, 0:64], psr)

        if dumps is not None:
            nc.sync.dma_start(out=dumps["qk"], in_=qk_sb)
            nc.sync.dma_start(out=dumps["v"], in_=v_sb)

        # prefetch W_o into SBUF (sync queue, ahead of phase-2 traffic)
        for m in range(HPC // 2):
            nc.sync.dma_start(out=wo_sb[m], in_=w_oT[128 * m : 128 * m + 128, :])

        # ============ phase 2+3: attention + output projection ============
        with ExitStack() as ph2:
            scpool = ph2.enter_context(tc.tile_pool(name="sc", bufs=2, space="PSUM"))
            pvpool = ph2.enter_context(tc.tile_pool(name="pv", bufs=1, space="PSUM"))
            ps3pool = ph2.enter_context(tc.tile_pool(name="ps3", bufs=2, space="PSUM"))
            expool = ph2.enter_context(tc.tile_pool(name="ex", bufs=8))
            npool = ph2.enter_context(tc.tile_pool(name="nrm", bufs=4))
            ypool = ph2.enter_context(tc.tile_pool(name="ysb", bufs=2))

            def blocks(lo, hi):
                while lo < hi:
                    b = min((lo // 512 + 1) * 512, hi)
                    yield lo, b
                    lo = b

            # software pipeline: scores for unit u+1 are emitted before the
            # exp/PV of unit u, so the PE fills ACT's exp latency.
            units = [
                (h, ih, jt)
                for ih in range(2)
                for h in range(HPC)
                for jt in range(8 * ih + 8)
            ]
            pv_state = {}

            def emit_scores(u):
                h, ih, jt = u
                i_lo, i_hi = 1024 * ih, 1024 * (ih + 1)
                j0 = 128 * jt
                c_lo = max(i_lo, j0)
                sc = scpool.tile([128, 1024], F32, tag="sc", name="sc")
                for lo, bhi in blocks(c_lo, i_hi):
                    nc.tensor.matmul(
                        sc[:, lo - i_lo : bhi - i_lo],
                        lhsT=kp[h][:, j0 : j0 + 128],
                        rhs=qk_sb[:, h // 2, lo:bhi],
                        start=True,
                        stop=True,
                    )
                return sc

            def emit_consume(u, sc):
                h, ih, jt = u
                i_lo, i_hi = 1024 * ih, 1024 * (ih + 1)
                j0 = 128 * jt
                diag = j0 >= i_lo
                c_lo = max(i_lo, j0)
                c_off = c_lo - i_lo
                if jt == 0:
                    pv_state[(h, ih)] = [
                        pvpool.tile([128, 512], F32, tag=f"pv{b}", name=f"pv{b}")
                        for b in range(2)
                    ]
                pv = pv_state[(h, ih)]
                ex = expool.tile([128, 1024], IN_DT, tag="ex", name="ex")
                nc.scalar.activation(
                    ex[:, c_off:1024],
                    sc[:, c_off:1024],
                    mybir.ActivationFunctionType.Exp,
                    bias=mb_sb[:, jt : jt + 1],
                    scale=SCALE,
                )
                if diag:
                    # causal boundary lives in the first 128 cols
                    nc.vector.tensor_mul(
                        ex[:, c_off : c_off + 128],
                        ex[:, c_off : c_off + 128],
                        cmask,
                    )
                # masked (diagonal) block last so only it waits on the mul
                blist = list(blocks(c_lo, i_hi))
                if diag and len(blist) > 1:
                    blist = blist[1:] + blist[:1]
                for lo, bhi in blist:
                    ib2 = (lo - i_lo) // 512
                    a0 = i_lo + 512 * ib2
                    nc.tensor.matmul(
                        pv[ib2][:, lo - a0 : bhi - a0],
                        lhsT=v_g[:, jt, h, :],
                        rhs=ex[:, lo - i_lo : bhi - i_lo],
                        start=(jt == 0),
                        stop=(jt == 4 * (2 * ih + ib2) + 3),
                    )
                for ib2 in range(2):
                    if jt == 4 * (2 * ih + ib2) + 3:
                        emit_normalize(h, ih, ib2)
                if jt == 8 * ih + 7:
                    pv_state.pop((h, ih))

            def emit_normalize(h, ih, ib2):
                i_lo = 1024 * ih
                pv = pv_state[(h, ih)]
                # normalize: O = PV / l (l on psum partition 64; DVE lanes are
                # partition-locked, gpsimd broadcast reads partition 0)
                if True:
                    if True:
                        if True:
                            acc = pv[ib2]
                            gl = i_lo + 512 * ib2
                            lsb = npool.tile([65, 512], F32, tag="lsb")
                            nc.vector.tensor_copy(lsb[64:65, :], acc[64:65, :])
                            l0 = npool.tile([1, 512], F32, tag="l0")
                            nc.sync.dma_start(out=l0, in_=lsb[64:65, :])
                            braw = npool.tile([64, 512], F32, tag="braw")
                            nc.gpsimd.partition_broadcast(braw, l0)
                            bc = npool.tile([64, 512], F32, tag="bc")
                            nc.vector.reciprocal_approx_fast(bc, braw)
                            if h % 2 == 0:
                                nc.vector.tensor_mul(
                                    o_pair[h // 2][0:64, gl : gl + 512],
                                    acc[0:64, :],
                                    bc,
                                )
                            else:
                                ot = npool.tile([64, 512], IN_DT, tag="ot")
                                nc.vector.tensor_mul(ot, acc[0:64, :], bc)
                                nc.sync.dma_start(
                                    out=o_pair[h // 2][64:128, gl : gl + 512], in_=ot
                                )

            ys_state = {}

            def emit_wo(tt, eb):
                if eb == 0:
                    ys_state[tt] = ypool.tile([128, DIM], F32, tag="ys", name="ys")
                ys = ys_state[tt]
                ps = ps3pool.tile([128, 512], F32, tag="ps3", name="ps3")
                for m in range(HPC // 2):
                    nc.tensor.matmul(
                        ps,
                        lhsT=o_pair[m][:, 128 * tt : 128 * tt + 128],
                        rhs=wo_sb[m][:, 512 * eb : 512 * eb + 512],
                        start=(m == 0),
                        stop=(m == HPC // 2 - 1),
                    )
                nc.vector.tensor_copy(ys[:, 512 * eb : 512 * eb + 512], ps)
                if eb == 1:
                    nc.sync.dma_start(
                        out=y[128 * tt : 128 * tt + 128, :], in_=ys_state.pop(tt)
                    )

            # pipeline: scores one unit ahead. Wo chains interleave into the
            # second query-half: tt 0-7 once all heads' first-half O is ready,
            # tt 8-11 into the last head's final units (its ib2=0 block
            # normalizes at jt=11), tt 12-15 after the loop.
            n_ih0 = HPC * 8
            wo_sched = {}  # unit index -> list of (tt, eb)
            ih1_idxs = list(range(n_ih0, len(units)))
            first_half_chains = [(tt, eb) for tt in range(8) for eb in range(2)]
            step = max(1, (len(ih1_idxs) - 4) // (len(first_half_chains) + 1))
            for ci, ch in enumerate(first_half_chains):
                idx = ih1_idxs[min((ci + 1) * step, len(ih1_idxs) - 5)]
                wo_sched.setdefault(idx, []).append(ch)
            late = [(tt, eb) for tt in range(8, 12) for eb in range(2)]
            for ci, ch in enumerate(late):
                idx = ih1_idxs[-4 + min(ci // 2, 3)]
                wo_sched.setdefault(idx, []).append(ch)

            sc_next = emit_scores(units[0])
            for i in range(len(units)):
                sc_cur = sc_next
                if i + 1 < len(units):
                    sc_next = emit_scores(units[i + 1])
                emit_consume(units[i], sc_cur)
                for tt, eb in wo_sched.get(i, ()):
                    emit_wo(tt, eb)

            for tt in range(12, JT):
                for eb in range(2):
                    emit_wo(tt, eb)

        if dumps is not None:
            for m in range(HPC // 2):
                nc.sync.dma_start(out=dumps["o"][:, m, :], in_=o_pair[m])


_PROGRAM_CACHE = {}


def build_program(debug_dump=False):
    key = ("nc", debug_dump)
    if key in _PROGRAM_CACHE:
        return _PROGRAM_CACHE[key]
    nc = bacc.Bacc(None, target_bir_lowering=False, debug=False)
    xT = nc.dram_tensor("xT", [DIM, S], IN_DT, kind="ExternalInput")
    w_qkT = nc.dram_tensor("w_qkT", [DIM, 2 * HPC * D], IN_DT, kind="ExternalInput")
    w_vT = nc.dram_tensor("w_vT", [DIM, HPC * D], IN_DT, kind="ExternalInput")
    w_oT = nc.dram_tensor("w_oT", [HPC * D, DIM], IN_DT, kind="ExternalInput")
    mask_bias = nc.dram_tensor("mask_bias", [128, JT], F32, kind="ExternalInput")
    y = nc.dram_tensor("y", [S, DIM], F32, kind="ExternalOutput")
    dumps = None
    if debug_dump:
        dumps = {
            "qk": nc.dram_tensor("qk_dump", [128, 8, S], IN_DT, kind="ExternalOutput")[:],
            "v": nc.dram_tensor("v_dump", [128, JT, HPC * 128], IN_DT, kind="ExternalOutput")[:],
            "o": nc.dram_tensor("o_dump", [128, HPC // 2, S], IN_DT, kind="ExternalOutput")[:],
        }
    with tile.TileContext(nc) as tc:
        _build_body(tc, xT[:], w_qkT[:], w_vT[:], w_oT[:], mask_bias[:], y[:], dumps)
    nc.compile()
    _PROGRAM_CACHE[key] = nc
    return nc


def make_in_maps(x, src_mask, W_qkv, W_o):
    import ml_dtypes

    np_in = ml_dtypes.bfloat16 if IN_DT == BF16 else np.float32
    x = np.asarray(x, dtype=np.float32)
    src_mask = np.asarray(src_mask)
    W_qkv = np.asarray(W_qkv, dtype=np.float32)
    W_o = np.asarray(W_o, dtype=np.float32)

    in_maps = []
    for c in range(N_CORES):
        b, g = c // GROUPS, c % GROUPS
        hw = HPC * D  # 512
        wq = W_qkv[g * hw : (g + 1) * hw]
        wk = W_qkv[DIM + g * hw : DIM + (g + 1) * hw]
        wv = W_qkv[2 * DIM + g * hw : 2 * DIM + (g + 1) * hw]
        mb = np.where(
            src_mask[b].reshape(JT, 128).T, np.float32(MASK_BIAS), np.float32(0.0)
        ).astype(np.float32)
        in_maps.append(
            {
                "xT": np.ascontiguousarray(x[b].T).astype(np_in),
                "w_qkT": np.ascontiguousarray(np.concatenate([wq, wk], 0).T).astype(
                    np_in
                ),
                "w_vT": np.ascontiguousarray(wv.T).astype(np_in),
                "w_oT": np.ascontiguousarray(
                    W_o[:, g * hw : (g + 1) * hw].T
                ).astype(np_in),
                "mask_bias": np.ascontiguousarray(mb),
            }
        )
    return in_maps


def run(x, src_mask, W_qkv, W_o, trace=False):
    nc = build_program()
    in_maps = make_in_maps(x, src_mask, W_qkv, W_o)
    res = run_bass_kernel_spmd(nc, in_maps, list(range(N_CORES)), trace=trace)
    parts = [res.results[c]["y"] for c in range(N_CORES)]
    out = np.empty((B, S, DIM), dtype=np.float32)
    for b in range(B):
        out[b] = parts[GROUPS * b] + parts[GROUPS * b + 1]
    return out, res


def kernel(x, src_mask, W_qkv, W_o):
    out, _ = run(x, src_mask, W_qkv, W_o, trace=False)
    return out
